# revision 61
# baseline (speedup 1.0000x reference)
"""InterpretableMultiHeadAttention on 8 Trainium2 NeuronCores.

Model (reference): qkv = x @ W_qkv; 16 q/k heads of 64, one shared v head;
causal softmax attention per head with shared V; mean over heads; @ W_out.

Sharding: core = (batch b, head-group hg of 8 heads). Each core computes its
8 heads' attention for its batch, applies (W_out/16) to the head-sum, and the
host adds the two head-group partials per batch.

Layout strategy (the TRN2 cost model charges matmuls per OUTPUT FREE ROW,
independent of K/M — so every matmul keeps its output free dim as small as
the math allows, and softmax probabilities are consumed as lhsT):
  qT/kT   [dh, t]  <- lhsT = W-slice [d, 128], rhs = xb [d, t]  (bf16, N=512)
  v       [t, dh]  <- lhsT = xb [d, t-tile],  rhs = wv [d, 64]  (bf16, N=64)
  scoresT [s, t]   <- lhsT = kT [dh, s-tile], rhs = qT [dh, w]  (bf16)
  expT    [s, t]   <- ONE exp activation per (head, s-row), w <= 1024; pairs
                      of narrow rows share one S tile (one per PSUM bank)
  causal diag mask <- Pool-engine affine_select zeroing expT below the diag
  pv      [t, dh]  <- lhsT = expT [s, t-tile], rhs = v [s, 64] (bf16, N=64:
                      2.3x less PE than the [dh, t] layout); softmax row-sums
                      accumulate beside it via N=1 ones-column matmuls
  normalize+head-sum on DVE: per-partition reciprocal + broadcast multiply
  out     [t, m]   <- lhsT = accT [dh, t-tile] (PE-transposed), rhs = W_out/16
Causality at 128-block granularity (136/256 blocks). The t-range is split in
two 1024-wide passes so each pair's pv/dn accumulators fit 3 PSUM banks (one
accumulation group per bank: a matmul `start` wipes its whole bank). The two
heads of a pair interleave row-by-row so PE fills the Act engine's exp
latency; the next pair's QKV projection trickles behind at lower priority.
No softmax max-subtraction: scores/8 ~ N(0,1) so exp is well-bounded.
"""

import numpy as np

import concourse.bass as bass
import concourse.mybir as mybir
import concourse.tile as tile
from concourse.bass_utils import run_bass_kernel_spmd
from concourse.masks import make_identity

F32 = mybir.dt.float32
BF = mybir.dt.bfloat16  # 1 cyc/row at any N

B, T, D = 4, 2048, 1024
H, DH = 16, 64
HPC = 8          # heads per core
NPAIR = HPC // 2
DCH = D // 128   # 8 contraction chunks
N_CORES = 8
_uid = [0]


def _split_multiwaits(nc, maxw=1):
    """walrus rejects instructions with multiple sync waits (observed on the
    Tile exit drain). Move extra waits onto same-engine NoOps just before."""
    for _name, bbh in nc.bb_map.items():
        bb = bbh.bb if hasattr(bbh, "bb") else bbh
        insts = bb.instructions
        new = []
        for inst in insts:
            si = inst.sync_info
            if si is not None and len(si.on_wait) > maxw:
                waits = list(si.on_wait)
                extra, keep = waits[:-maxw], waits[-maxw:]
                for k in range(0, len(extra), maxw):
                    _uid[0] += 1
                    nop = mybir.InstNoOp(
                        name=f"I-waitsplit-{_uid[0]}", ins=[], outs=[]
                    )
                    nop.engine = inst.engine
                    nop.sync_info = mybir.SyncInfo(
                        on_wait=extra[k : k + maxw], on_update=[]
                    )
                    new.append(nop)
                inst.sync_info = mybir.SyncInfo(
                    on_wait=keep, on_update=list(si.on_update)
                )
            new.append(inst)
        insts[:] = new


def _emit_body(nc, tc, xb, wqk, wv, wout, out):
    Exp = mybir.ActivationFunctionType.Exp
    ts = bass.ts

    from contextlib import ExitStack

    ctx = ExitStack()
    consts = ctx.enter_context(tc.tile_pool(name="consts", bufs=1))
    ident_f = consts.tile([128, 128], F32)
    make_identity(nc, ident_f)
    wout_sb = consts.tile([64, 1024], BF)
    nc.gpsimd.dma_start(out=wout_sb, in_=wout[:])
    v_sb = consts.tile([128, 16, 65], BF)   # v tiles [t,dh] + ones column
    nc.vector.memset(v_sb[:, :, 64:65], 1.0)
    qk_sb = consts.tile([128, 8, T], BF)    # groups: q-pairs 0..3, k-pairs 4..7
    acc = consts.tile([128, 16, 64], F32)   # sum_h attn_h/denom_h, [t, dh]
    nc.vector.memset(acc, 0.0)
    accT = consts.tile([64, 16, 128], BF)
    ot_pool = ctx.enter_context(tc.tile_pool(name="ot", bufs=6))

    sb_ctx = ExitStack()
    xt_pool = sb_ctx.enter_context(tc.tile_pool(name="xt", bufs=1, side="right"))
    w_pool = sb_ctx.enter_context(tc.tile_pool(name="w", bufs=1, side="right"))
    xb_sb = xt_pool.tile([128, DCH, T], BF)
    wqk_sb = w_pool.tile([128, DCH, 1024], BF)
    wv_sb = w_pool.tile([128, DCH, 64], BF)
    # wqk is packed pair-major on the host: pair m at cols [256m, 256m+256)
    # (q then k). Inputs arrive as a few large HWDGE transfers (Pool-issued
    # per-chunk DMAs pay ~1us software-DGE serialization each); pair 0's wqk
    # slice goes ahead of the rest so the first pair unblocks right after xb.
    def r3(ap, p=128):
        return ap.rearrange("(dc p) c -> p dc c", p=p)

    nc.scalar.dma_start(out=wv_sb, in_=r3(wv[:]))
    nc.scalar.dma_start(out=xb_sb[:, 0:4, :], in_=r3(xb[0:512, :]))
    nc.sync.dma_start(out=wqk_sb[:, :, 0:256], in_=r3(wqk[:, 0:256]))
    nc.scalar.dma_start(out=xb_sb[:, 4:8, :], in_=r3(xb[512:1024, :]))
    # The non-pair-0 weights must not cut ahead of xb on the DMA engines
    # (xb gates everything). A tiny Pool copy reading xb into the transfer's
    # destination region forces the DMA (WAW) to request only after xb lands.
    nc.gpsimd.tensor_copy(wqk_sb[:, 7:8, 256:257], xb_sb[:, 7:8, 2047:2048])
    nc.gpsimd.dma_start(out=wqk_sb[:, :, 256:1024], in_=r3(wqk[:, 256:1024]))

    qkctx = ExitStack()
    psqk = qkctx.enter_context(
        tc.tile_pool(name="psqk", bufs=1, space="PSUM", side="right")
    )

    qk0ctx = ExitStack()
    psqk0 = qk0ctx.enter_context(
        tc.tile_pool(name="psqk0", bufs=3, space="PSUM", side="right")
    )

    # ---- v projection: out[t, dh], all 16 t-tiles chained per dc.
    # One accumulation group per PSUM bank (a start wipes the whole bank), so
    # start only on the first touch of each bank, stop on the last.
    psv_ctx = ExitStack()
    psv = psv_ctx.enter_context(
        tc.tile_pool(name="psv", bufs=1, space="PSUM", side="right")
    )
    v_ps = psv.tile([128, 16, 64], F32)

    def emit_vproj(dcs):
        for dc in dcs:
            for tt in range(16):
                nc.tensor.matmul(
                    v_ps[:, tt, :],
                    xb_sb[:, dc, ts(tt, 128)],
                    wv_sb[:, dc, :],
                    start=(dc == 0 and tt % 8 == 0),
                    stop=(dc == DCH - 1 and tt % 8 == 7),
                )



    def emit_qkproj(m, pool=None, use_act=False, tccs=range(4)):
        # project q-pair m (group m) and k-pair m (group 4+m) -> qk_sb bf16.
        # q/k alternate per t-chunk so the first attention rows unblock early.
        # Pair 0 runs before any exp, so its copies can also use the idle Act
        # engine to halve the copy serialization on the critical path.
        for tcc in tccs:
            for g, off in ((m, 256 * m), (4 + m, 256 * m + 128)):
                ps = (pool or psqk).tile([128, 512], F32, tag="qk")
                for dc in range(DCH):
                    nc.tensor.matmul(
                        ps,
                        wqk_sb[:, dc, off : off + 128],
                        xb_sb[:, dc, ts(tcc, 512)],
                        start=(dc == 0),
                        stop=(dc == DCH - 1),
                    )
                if use_act:
                    nc.scalar.copy(qk_sb[:, g, ts(tcc, 512)], ps)
                else:
                    nc.vector.tensor_copy(qk_sb[:, g, ts(tcc, 512)], ps)

    emit_vproj(range(DCH))

    emit_qkproj(0, pool=psqk0, use_act=True)
    # v copy emitted after pair-0's q copies so DVE's in-order stream
    # doesn't head-of-line block the attention-critical copies behind it
    nc.vector.tensor_copy(v_sb[:, :, 0:64], v_ps)
    psv_ctx.close()
    qk0ctx.close()

    at_ctx = ExitStack()
    psS = at_ctx.enter_context(tc.tile_pool(name="psS", bufs=2, space="PSUM"))
    psPV = at_ctx.enter_context(tc.tile_pool(name="psPV", bufs=1, space="PSUM"))
    et_pool = at_ctx.enter_context(tc.tile_pool(name="et", bufs=8))
    nrm_pool = at_ctx.enter_context(tc.tile_pool(name="nrm", bufs=2))

    def emit_attn_pass(m, pass_):
        # The two heads of the pair interleave at row granularity: while the
        # Act engine exponentiates head A's row, PE runs head B's scores/PV.
        # pv holds both heads' accumulators (one bank per head = one
        # accumulation group each); dn shares a single bank as ONE group
        # spanning both heads (disjoint slices).
        if True:
            t0 = 1024 * pass_
            jb = 8 * pass_
            imax = 8 + 8 * pass_
            pv = psPV.tile([128, 2, 8, 64], F32, tag="pv")
            dn = psPV.tile([128, 2, 8], F32, tag="dn")

            def emit_pv_row(ets, i, t_start, off):
                # PV emission lags one row group behind S/exp: keeps PE's
                # in-order stream from head-of-line blocking on the pv slot
                # (only freed by the previous pass's normalize) or on exp(i).
                first = i == 0
                last = i == imax - 1
                for hh in (0, 1):
                    for jj in range(max(i, jb), jb + 8):
                        lo = off + 128 * jj - t_start
                        eti = ets[hh][:, lo : lo + 128]
                        st = first and jj == jb
                        nc.tensor.matmul(
                            pv[:, hh, jj - jb, :],
                            eti,
                            v_sb[:, i, 0:64],
                            start=st,
                            stop=last,
                        )
                        nc.tensor.matmul(
                            dn[:, hh, jj - jb : jj - jb + 1],
                            eti,
                            v_sb[:, i, 64:65],
                            start=(st and hh == 0),
                            stop=(last and hh == 1),
                        )

            def emit_srow(S, et_w, i, t_start, w, off, own, fill_to):
                # scores for row i into S[:, off:off+w]; each PSUM bank of the
                # tile holds exactly one accumulation group. `own` = this row
                # owns bank0 of the tile starting at off=0 (w may span both
                # banks); merged rows sit alone in bank1 (off=512, w<=512).
                lim = min(w, 512)
                p0 = 64 * own
                kTi = qk_sb[p0 : p0 + 64, 4 + m, ts(i, 128)]
                qT = qk_sb[p0 : p0 + 64, m, :]
                junk = fill_to - lim if off == 0 else 0
                nc.tensor.matmul(
                    S[:, off : off + lim],
                    kTi,
                    qT[:, t_start : t_start + lim],
                    start=True,
                    stop=(junk == 0),
                )
                if junk:
                    # merged tiles: pad bank0 to its full span with junk
                    # scores (same accumulation group; exp'd, never read)
                    nc.tensor.matmul(
                        S[:, off + lim : off + fill_to],
                        kTi,
                        qT[:, 0:junk],
                        start=False,
                        stop=True,
                    )
                if w > lim:
                    nc.tensor.matmul(
                        S[:, off + 512 : off + w],
                        kTi,
                        qT[:, t_start + 512 : t_start + w],
                        start=True,
                        stop=True,
                    )

            # rows with w <= 512 pair up two-per-S-tile (one per bank) so a
            # single exp instruction covers both; pairing (a,a+2) keeps the
            # wider row in bank0 with zero wasted exp work.
            if pass_ == 0:
                groups = [[0], [1], [2], [3], [4, 6], [5, 7]]
            else:
                groups = [[i] for i in range(12)] + [[12, 14], [13, 15]]
            pend = []
            for grp in groups:
                rows = []
                for r_idx, i in enumerate(grp):
                    t_start = max(128 * i, t0)
                    rows.append((i, t_start, t0 + 1024 - t_start, 512 * r_idx))
                ets = []
                for hh in (0, 1):
                    S = psS.tile([128, 1024], F32, tag="s")
                    et = et_pool.tile([128, 1024], BF, tag="et")
                    for i, t_start, w, off in rows:
                        ft = min(w, 512) if len(rows) == 1 else 512
                        emit_srow(S, et, i, t_start, w, off, hh, ft)
                    wtot = rows[-1][3] + rows[-1][2]
                    nc.scalar.activation(
                        et[:, 0:wtot], S[:, 0:wtot], Exp, scale=0.125
                    )
                    for i, t_start, w, off in rows:
                        if 128 * i >= t0:
                            # causal diag: zero et where col < row, on the
                            # otherwise-idle Pool engine instead of a PE
                            # additive-mask matmul
                            nc.gpsimd.affine_select(
                                out=et[:, off : off + 128],
                                in_=et[:, off : off + 128],
                                compare_op=mybir.AluOpType.is_ge,
                                fill=0.0,
                                base=0,
                                # keep (col - row) >= 0
                                pattern=[[1, 128]],
                                channel_multiplier=-1,
                            )
                    ets.append(et)
                new = [(ets, i, t_start, off) for i, t_start, w, off in rows]
                for p in pend:
                    emit_pv_row(*p)
                pend = new
            for p in pend:
                emit_pv_row(*p)
            # normalize by the row-sums; accumulate both heads into acc
            rcp = nrm_pool.tile([128, 2, 8], F32, tag="rcp")
            nc.vector.reciprocal(rcp, dn)
            prod = nrm_pool.tile([128, 2, 8, 64], F32, tag="prod")
            nc.vector.tensor_mul(
                prod, pv, rcp.unsqueeze(3).broadcast_to([128, 2, 8, 64])
            )
            sum2 = nrm_pool.tile([128, 8, 64], F32, tag="sum2")
            nc.vector.tensor_add(sum2, prod[:, 0], prod[:, 1])
            nc.vector.tensor_add(
                acc[:, jb : jb + 8, :], acc[:, jb : jb + 8, :], sum2
            )

    def emit_stageD_tt(tt, psD, act_copy):
        pT = psD.tile([64, 128], F32, tag="d")
        nc.tensor.transpose(pT, acc[:, tt, :], ident_f)
        nc.vector.tensor_copy(accT[:, tt, :], pT)
        ot = ot_pool.tile([128, 1024], BF, tag="ot")
        for mc in range(2):
            po = psD.tile([128, 512], F32, tag="d")
            nc.tensor.matmul(
                po,
                accT[:, tt, :],
                wout_sb[:, ts(mc, 512)],
                start=True,
                stop=True,
            )
            if mc == 1 and act_copy:
                nc.scalar.copy(ot[:, 512:1024], po)
            else:
                nc.vector.tensor_copy(ot[:, ts(mc, 512)], po)
        nc.sync.dma_start(out=out[ts(tt, 128), :], in_=ot)

    dA_ctx = ExitStack()
    for m in range(NPAIR):
        emit_attn_pass(m, 0)
        if m + 1 == NPAIR:
            # stage D for the first t-half trickles through the last pair's
            # pass B on the bank freed by psqk (single slot self-throttles).
            qkctx.close()
            sb_ctx.close()
            psD_a = dA_ctx.enter_context(
                tc.tile_pool(name="psDa", bufs=1, space="PSUM", side="right")
            )
        emit_attn_pass(m, 1)
        if m + 1 < NPAIR:
            emit_qkproj(m + 1)
        if m + 1 == NPAIR:
            for tt in range(8):
                emit_stageD_tt(tt, psD_a, act_copy=False)
    dA_ctx.close()
    at_ctx.close()

    # ---- stage D, second t-half: runs at the tail with roomy pools ----
    d_ctx = ExitStack()
    psD_b = d_ctx.enter_context(tc.tile_pool(name="psDb", bufs=8, space="PSUM"))
    for tt in range(8, 16):
        emit_stageD_tt(tt, psD_b, act_copy=True)
    d_ctx.close()
    ctx.close()


_NC_CACHE = [None]


def build_nc():
    if _NC_CACHE[0] is not None:
        return _NC_CACHE[0]
    nc = bass.Bass("TRN2", target_bir_lowering=False, debug=False)
    xb = nc.declare_dram_parameter("xb", [D, T], BF, isOutput=False)
    wqk = nc.declare_dram_parameter("wqk", [D, 1024], BF, isOutput=False)
    wv = nc.declare_dram_parameter("wv", [D, 64], BF, isOutput=False)
    wout = nc.declare_dram_parameter("wout", [64, D], BF, isOutput=False)
    out = nc.declare_dram_parameter("out", [T, D], BF, isOutput=True)
    with tile.TileContext(nc) as tc, nc.allow_low_precision(
        reason="f32r qkv proj + bf16 attention path; ~6e-3 rel err vs fp32 ref"
    ):
        _emit_body(nc, tc, xb, wqk, wv, wout, out)
    _split_multiwaits(nc, maxw=1)
    _NC_CACHE[0] = nc
    return nc


def make_in_maps(x, W_qkv, W_out):
    import ml_dtypes

    bf16 = ml_dtypes.bfloat16
    wv = np.ascontiguousarray(W_qkv[:, 2 * H * DH :]).astype(bf16)
    wout = np.ascontiguousarray(np.asarray(W_out) / float(H)).astype(bf16)
    in_maps = []
    for core in range(N_CORES):
        b, hg = core // 2, core % 2
        xTf = np.ascontiguousarray(np.asarray(x[b]).T, dtype=np.float32)
        cols = []
        for mp in range(NPAIR):  # pair-major: [q-pair | k-pair] per pair
            h0 = hg * HPC + 2 * mp
            for off in (0, H * DH):
                cols.append(W_qkv[:, off + h0 * DH : off + (h0 + 2) * DH])
        wqk = np.ascontiguousarray(np.concatenate(cols, axis=1)).astype(bf16)
        in_maps.append(
            {
                "xb": xTf.astype(bf16),
                "wqk": wqk,
                "wv": wv,
                "wout": wout,
            }
        )
    return in_maps


def kernel(x, W_qkv, W_out, _trace=False, _trace_kwargs=None):
    nc = build_nc()
    in_maps = make_in_maps(x, W_qkv, W_out)
    res = run_bass_kernel_spmd(
        nc, in_maps, list(range(N_CORES)), trace=_trace, **(_trace_kwargs or {})
    )
    out = np.empty((B, T, D), dtype=np.float32)
    for b in range(B):
        out[b] = np.asarray(res.results[2 * b]["out"], dtype=np.float32) + np.asarray(res.results[2 * b + 1]["out"], dtype=np.float32)
    if _trace:
        return out, res
    return out


# revision 62
# speedup vs baseline: 1.0013x; 1.0013x over previous
"""InterpretableMultiHeadAttention on 8 Trainium2 NeuronCores.

Model (reference): qkv = x @ W_qkv; 16 q/k heads of 64, one shared v head;
causal softmax attention per head with shared V; mean over heads; @ W_out.

Sharding: core = (batch b, head-group hg of 8 heads). Each core computes its
8 heads' attention for its batch, applies (W_out/16) to the head-sum, and the
host adds the two head-group partials per batch.

Layout strategy (the TRN2 cost model charges matmuls per OUTPUT FREE ROW,
independent of K/M — so every matmul keeps its output free dim as small as
the math allows, and softmax probabilities are consumed as lhsT):
  qT/kT   [dh, t]  <- lhsT = W-slice [d, 128], rhs = xb [d, t]  (bf16, N=512)
  v       [t, dh]  <- lhsT = xb [d, t-tile],  rhs = wv [d, 64]  (bf16, N=64)
  scoresT [s, t]   <- lhsT = kT [dh, s-tile], rhs = qT [dh, w]  (bf16)
  expT    [s, t]   <- ONE exp activation per (head, s-row), w <= 1024; pairs
                      of narrow rows share one S tile (one per PSUM bank)
  causal diag mask <- Pool-engine affine_select zeroing expT below the diag
  pv      [t, dh]  <- lhsT = expT [s, t-tile], rhs = v [s, 64] (bf16, N=64:
                      2.3x less PE than the [dh, t] layout); softmax row-sums
                      accumulate beside it via N=1 ones-column matmuls
  normalize+head-sum on DVE: per-partition reciprocal + broadcast multiply
  out     [t, m]   <- lhsT = accT [dh, t-tile] (PE-transposed), rhs = W_out/16
Causality at 128-block granularity (136/256 blocks). The t-range is split in
two 1024-wide passes so each pair's pv/dn accumulators fit 3 PSUM banks (one
accumulation group per bank: a matmul `start` wipes its whole bank). The two
heads of a pair interleave row-by-row so PE fills the Act engine's exp
latency; the next pair's QKV projection trickles behind at lower priority.
No softmax max-subtraction: scores/8 ~ N(0,1) so exp is well-bounded.
"""

import numpy as np

import concourse.bass as bass
import concourse.mybir as mybir
import concourse.tile as tile
from concourse.bass_utils import run_bass_kernel_spmd
from concourse.masks import make_identity

F32 = mybir.dt.float32
BF = mybir.dt.bfloat16  # 1 cyc/row at any N

B, T, D = 4, 2048, 1024
H, DH = 16, 64
HPC = 8          # heads per core
NPAIR = HPC // 2
DCH = D // 128   # 8 contraction chunks
N_CORES = 8
_uid = [0]


def _split_multiwaits(nc, maxw=1):
    """walrus rejects instructions with multiple sync waits (observed on the
    Tile exit drain). Move extra waits onto same-engine NoOps just before."""
    for _name, bbh in nc.bb_map.items():
        bb = bbh.bb if hasattr(bbh, "bb") else bbh
        insts = bb.instructions
        new = []
        for inst in insts:
            si = inst.sync_info
            if si is not None and len(si.on_wait) > maxw:
                waits = list(si.on_wait)
                extra, keep = waits[:-maxw], waits[-maxw:]
                for k in range(0, len(extra), maxw):
                    _uid[0] += 1
                    nop = mybir.InstNoOp(
                        name=f"I-waitsplit-{_uid[0]}", ins=[], outs=[]
                    )
                    nop.engine = inst.engine
                    nop.sync_info = mybir.SyncInfo(
                        on_wait=extra[k : k + maxw], on_update=[]
                    )
                    new.append(nop)
                inst.sync_info = mybir.SyncInfo(
                    on_wait=keep, on_update=list(si.on_update)
                )
            new.append(inst)
        insts[:] = new


def _emit_body(nc, tc, xb, wqk, wv, wout, out):
    Exp = mybir.ActivationFunctionType.Exp
    ts = bass.ts

    from contextlib import ExitStack

    ctx = ExitStack()
    consts = ctx.enter_context(tc.tile_pool(name="consts", bufs=1))
    ident_f = consts.tile([128, 128], F32)
    make_identity(nc, ident_f)
    wout_sb = consts.tile([64, 1024], BF)
    nc.gpsimd.dma_start(out=wout_sb, in_=wout[:])
    v_sb = consts.tile([128, 16, 65], BF)   # v tiles [t,dh] + ones column
    nc.vector.memset(v_sb[:, :, 64:65], 1.0)
    qk_sb = consts.tile([128, 8, T], BF)    # groups: q-pairs 0..3, k-pairs 4..7
    acc = consts.tile([128, 16, 64], F32)   # sum_h attn_h/denom_h, [t, dh]
    nc.vector.memset(acc, 0.0)
    accT = consts.tile([64, 16, 128], BF)
    ot_pool = ctx.enter_context(tc.tile_pool(name="ot", bufs=8))

    sb_ctx = ExitStack()
    xt_pool = sb_ctx.enter_context(tc.tile_pool(name="xt", bufs=1, side="right"))
    w_pool = sb_ctx.enter_context(tc.tile_pool(name="w", bufs=1, side="right"))
    xb_sb = xt_pool.tile([128, DCH, T], BF)
    wqk_sb = w_pool.tile([128, DCH, 1024], BF)
    wv_sb = w_pool.tile([128, DCH, 64], BF)
    # wqk is packed pair-major on the host: pair m at cols [256m, 256m+256)
    # (q then k). Inputs arrive as a few large HWDGE transfers (Pool-issued
    # per-chunk DMAs pay ~1us software-DGE serialization each); pair 0's wqk
    # slice goes ahead of the rest so the first pair unblocks right after xb.
    def r3(ap, p=128):
        return ap.rearrange("(dc p) c -> p dc c", p=p)

    nc.scalar.dma_start(out=wv_sb, in_=r3(wv[:]))
    nc.scalar.dma_start(out=xb_sb[:, 0:4, :], in_=r3(xb[0:512, :]))
    nc.sync.dma_start(out=wqk_sb[:, :, 0:256], in_=r3(wqk[:, 0:256]))
    nc.scalar.dma_start(out=xb_sb[:, 4:8, :], in_=r3(xb[512:1024, :]))
    # The non-pair-0 weights must not cut ahead of xb on the DMA engines
    # (xb gates everything). A tiny Pool copy reading xb into the transfer's
    # destination region forces the DMA (WAW) to request only after xb lands.
    nc.gpsimd.tensor_copy(wqk_sb[:, 7:8, 256:257], xb_sb[:, 7:8, 2047:2048])
    nc.gpsimd.dma_start(out=wqk_sb[:, :, 256:1024], in_=r3(wqk[:, 256:1024]))

    qkctx = ExitStack()
    psqk = qkctx.enter_context(
        tc.tile_pool(name="psqk", bufs=1, space="PSUM", side="right")
    )

    qk0ctx = ExitStack()
    psqk0 = qk0ctx.enter_context(
        tc.tile_pool(name="psqk0", bufs=3, space="PSUM", side="right")
    )

    # ---- v projection: out[t, dh], all 16 t-tiles chained per dc.
    # One accumulation group per PSUM bank (a start wipes the whole bank), so
    # start only on the first touch of each bank, stop on the last.
    psv_ctx = ExitStack()
    psv = psv_ctx.enter_context(
        tc.tile_pool(name="psv", bufs=1, space="PSUM", side="right")
    )
    v_ps = psv.tile([128, 16, 64], F32)

    def emit_vproj(dcs):
        for dc in dcs:
            for tt in range(16):
                nc.tensor.matmul(
                    v_ps[:, tt, :],
                    xb_sb[:, dc, ts(tt, 128)],
                    wv_sb[:, dc, :],
                    start=(dc == 0 and tt % 8 == 0),
                    stop=(dc == DCH - 1 and tt % 8 == 7),
                )



    def emit_qkproj(m, pool=None, use_act=False, tccs=range(4)):
        # project q-pair m (group m) and k-pair m (group 4+m) -> qk_sb bf16.
        # q/k alternate per t-chunk so the first attention rows unblock early.
        # Pair 0 runs before any exp, so its copies can also use the idle Act
        # engine to halve the copy serialization on the critical path.
        for tcc in tccs:
            for g, off in ((m, 256 * m), (4 + m, 256 * m + 128)):
                ps = (pool or psqk).tile([128, 512], F32, tag="qk")
                for dc in range(DCH):
                    nc.tensor.matmul(
                        ps,
                        wqk_sb[:, dc, off : off + 128],
                        xb_sb[:, dc, ts(tcc, 512)],
                        start=(dc == 0),
                        stop=(dc == DCH - 1),
                    )
                if use_act:
                    nc.scalar.copy(qk_sb[:, g, ts(tcc, 512)], ps)
                else:
                    nc.vector.tensor_copy(qk_sb[:, g, ts(tcc, 512)], ps)

    emit_vproj(range(DCH))

    emit_qkproj(0, pool=psqk0, use_act=True)
    # v copy emitted after pair-0's q copies so DVE's in-order stream
    # doesn't head-of-line block the attention-critical copies behind it
    nc.vector.tensor_copy(v_sb[:, :, 0:64], v_ps)
    psv_ctx.close()
    qk0ctx.close()

    at_ctx = ExitStack()
    psS = at_ctx.enter_context(tc.tile_pool(name="psS", bufs=2, space="PSUM"))
    psPV = at_ctx.enter_context(tc.tile_pool(name="psPV", bufs=1, space="PSUM"))
    et_pool = at_ctx.enter_context(tc.tile_pool(name="et", bufs=8))
    nrm_pool = at_ctx.enter_context(tc.tile_pool(name="nrm", bufs=2))

    def emit_attn_pass(m, pass_):
        # The two heads of the pair interleave at row granularity: while the
        # Act engine exponentiates head A's row, PE runs head B's scores/PV.
        # pv holds both heads' accumulators (one bank per head = one
        # accumulation group each); dn shares a single bank as ONE group
        # spanning both heads (disjoint slices).
        if True:
            t0 = 1024 * pass_
            jb = 8 * pass_
            imax = 8 + 8 * pass_
            pv = psPV.tile([128, 2, 8, 64], F32, tag="pv")
            dn = psPV.tile([128, 2, 8], F32, tag="dn")

            def emit_pv_row(ets, i, t_start, off):
                # PV emission lags one row group behind S/exp: keeps PE's
                # in-order stream from head-of-line blocking on the pv slot
                # (only freed by the previous pass's normalize) or on exp(i).
                first = i == 0
                last = i == imax - 1
                for hh in (0, 1):
                    for jj in range(max(i, jb), jb + 8):
                        lo = off + 128 * jj - t_start
                        eti = ets[hh][:, lo : lo + 128]
                        st = first and jj == jb
                        nc.tensor.matmul(
                            pv[:, hh, jj - jb, :],
                            eti,
                            v_sb[:, i, 0:64],
                            start=st,
                            stop=last,
                        )
                        nc.tensor.matmul(
                            dn[:, hh, jj - jb : jj - jb + 1],
                            eti,
                            v_sb[:, i, 64:65],
                            start=(st and hh == 0),
                            stop=(last and hh == 1),
                        )

            def emit_srow(S, et_w, i, t_start, w, off, own, fill_to):
                # scores for row i into S[:, off:off+w]; each PSUM bank of the
                # tile holds exactly one accumulation group. `own` = this row
                # owns bank0 of the tile starting at off=0 (w may span both
                # banks); merged rows sit alone in bank1 (off=512, w<=512).
                lim = min(w, 512)
                p0 = 64 * own
                kTi = qk_sb[p0 : p0 + 64, 4 + m, ts(i, 128)]
                qT = qk_sb[p0 : p0 + 64, m, :]
                junk = fill_to - lim if off == 0 else 0
                nc.tensor.matmul(
                    S[:, off : off + lim],
                    kTi,
                    qT[:, t_start : t_start + lim],
                    start=True,
                    stop=(junk == 0),
                )
                if junk:
                    # merged tiles: pad bank0 to its full span with junk
                    # scores (same accumulation group; exp'd, never read)
                    nc.tensor.matmul(
                        S[:, off + lim : off + fill_to],
                        kTi,
                        qT[:, 0:junk],
                        start=False,
                        stop=True,
                    )
                if w > lim:
                    nc.tensor.matmul(
                        S[:, off + 512 : off + w],
                        kTi,
                        qT[:, t_start + 512 : t_start + w],
                        start=True,
                        stop=True,
                    )

            # rows with w <= 512 pair up two-per-S-tile (one per bank) so a
            # single exp instruction covers both; pairing (a,a+2) keeps the
            # wider row in bank0 with zero wasted exp work.
            if pass_ == 0:
                groups = [[0], [1], [2], [3], [4, 6], [5, 7]]
            else:
                groups = [[i] for i in range(12)] + [[12, 14], [13, 15]]
            pend = []
            for grp in groups:
                rows = []
                for r_idx, i in enumerate(grp):
                    t_start = max(128 * i, t0)
                    rows.append((i, t_start, t0 + 1024 - t_start, 512 * r_idx))
                ets = []
                for hh in (0, 1):
                    S = psS.tile([128, 1024], F32, tag="s")
                    et = et_pool.tile([128, 1024], BF, tag="et")
                    for i, t_start, w, off in rows:
                        ft = min(w, 512) if len(rows) == 1 else 512
                        emit_srow(S, et, i, t_start, w, off, hh, ft)
                    wtot = rows[-1][3] + rows[-1][2]
                    nc.scalar.activation(
                        et[:, 0:wtot], S[:, 0:wtot], Exp, scale=0.125
                    )
                    for i, t_start, w, off in rows:
                        if 128 * i >= t0:
                            # causal diag: zero et where col < row, on the
                            # otherwise-idle Pool engine instead of a PE
                            # additive-mask matmul
                            nc.gpsimd.affine_select(
                                out=et[:, off : off + 128],
                                in_=et[:, off : off + 128],
                                compare_op=mybir.AluOpType.is_ge,
                                fill=0.0,
                                base=0,
                                # keep (col - row) >= 0
                                pattern=[[1, 128]],
                                channel_multiplier=-1,
                            )
                    ets.append(et)
                new = [(ets, i, t_start, off) for i, t_start, w, off in rows]
                for p in pend:
                    emit_pv_row(*p)
                pend = new
            for p in pend:
                emit_pv_row(*p)
            # normalize by the row-sums; accumulate both heads into acc
            rcp = nrm_pool.tile([128, 2, 8], F32, tag="rcp")
            nc.vector.reciprocal(rcp, dn)
            prod = nrm_pool.tile([128, 2, 8, 64], F32, tag="prod")
            nc.vector.tensor_mul(
                prod, pv, rcp.unsqueeze(3).broadcast_to([128, 2, 8, 64])
            )
            sum2 = nrm_pool.tile([128, 8, 64], F32, tag="sum2")
            nc.vector.tensor_add(sum2, prod[:, 0], prod[:, 1])
            nc.vector.tensor_add(
                acc[:, jb : jb + 8, :], acc[:, jb : jb + 8, :], sum2
            )

    def emit_stageD_tt(tt, psD, act_copy):
        pT = psD.tile([64, 128], F32, tag="d")
        nc.tensor.transpose(pT, acc[:, tt, :], ident_f)
        nc.vector.tensor_copy(accT[:, tt, :], pT)
        ot = ot_pool.tile([128, 1024], BF, tag="ot")
        for mc in range(2):
            po = psD.tile([128, 512], F32, tag="d")
            nc.tensor.matmul(
                po,
                accT[:, tt, :],
                wout_sb[:, ts(mc, 512)],
                start=True,
                stop=True,
            )
            if mc == 1 and act_copy:
                nc.scalar.copy(ot[:, 512:1024], po)
            else:
                nc.vector.tensor_copy(ot[:, ts(mc, 512)], po)
        nc.sync.dma_start(out=out[ts(tt, 128), :], in_=ot)

    dA_ctx = ExitStack()
    for m in range(NPAIR):
        emit_attn_pass(m, 0)
        if m + 1 == NPAIR:
            # stage D for the first t-half trickles through the last pair's
            # pass B on the bank freed by psqk (single slot self-throttles).
            qkctx.close()
            sb_ctx.close()
            psD_a = dA_ctx.enter_context(
                tc.tile_pool(name="psDa", bufs=1, space="PSUM", side="right")
            )
        emit_attn_pass(m, 1)
        if m + 1 < NPAIR:
            emit_qkproj(m + 1)
        if m + 1 == NPAIR:
            for tt in range(8):
                emit_stageD_tt(tt, psD_a, act_copy=False)
    dA_ctx.close()
    at_ctx.close()

    # ---- stage D, second t-half: runs at the tail with roomy pools ----
    d_ctx = ExitStack()
    psD_b = d_ctx.enter_context(tc.tile_pool(name="psDb", bufs=8, space="PSUM"))
    for tt in range(8, 16):
        emit_stageD_tt(tt, psD_b, act_copy=True)
    d_ctx.close()
    ctx.close()


_NC_CACHE = [None]


def build_nc():
    if _NC_CACHE[0] is not None:
        return _NC_CACHE[0]
    nc = bass.Bass("TRN2", target_bir_lowering=False, debug=False)
    xb = nc.declare_dram_parameter("xb", [D, T], BF, isOutput=False)
    wqk = nc.declare_dram_parameter("wqk", [D, 1024], BF, isOutput=False)
    wv = nc.declare_dram_parameter("wv", [D, 64], BF, isOutput=False)
    wout = nc.declare_dram_parameter("wout", [64, D], BF, isOutput=False)
    out = nc.declare_dram_parameter("out", [T, D], BF, isOutput=True)
    with tile.TileContext(nc) as tc, nc.allow_low_precision(
        reason="f32r qkv proj + bf16 attention path; ~6e-3 rel err vs fp32 ref"
    ):
        _emit_body(nc, tc, xb, wqk, wv, wout, out)
    _split_multiwaits(nc, maxw=1)
    _NC_CACHE[0] = nc
    return nc


def make_in_maps(x, W_qkv, W_out):
    import ml_dtypes

    bf16 = ml_dtypes.bfloat16
    wv = np.ascontiguousarray(W_qkv[:, 2 * H * DH :]).astype(bf16)
    wout = np.ascontiguousarray(np.asarray(W_out) / float(H)).astype(bf16)
    in_maps = []
    for core in range(N_CORES):
        b, hg = core // 2, core % 2
        xTf = np.ascontiguousarray(np.asarray(x[b]).T, dtype=np.float32)
        cols = []
        for mp in range(NPAIR):  # pair-major: [q-pair | k-pair] per pair
            h0 = hg * HPC + 2 * mp
            for off in (0, H * DH):
                cols.append(W_qkv[:, off + h0 * DH : off + (h0 + 2) * DH])
        wqk = np.ascontiguousarray(np.concatenate(cols, axis=1)).astype(bf16)
        in_maps.append(
            {
                "xb": xTf.astype(bf16),
                "wqk": wqk,
                "wv": wv,
                "wout": wout,
            }
        )
    return in_maps


def kernel(x, W_qkv, W_out, _trace=False, _trace_kwargs=None):
    nc = build_nc()
    in_maps = make_in_maps(x, W_qkv, W_out)
    res = run_bass_kernel_spmd(
        nc, in_maps, list(range(N_CORES)), trace=_trace, **(_trace_kwargs or {})
    )
    out = np.empty((B, T, D), dtype=np.float32)
    for b in range(B):
        out[b] = np.asarray(res.results[2 * b]["out"], dtype=np.float32) + np.asarray(res.results[2 * b + 1]["out"], dtype=np.float32)
    if _trace:
        return out, res
    return out


# revision 63
# speedup vs baseline: 1.0022x; 1.0010x over previous
"""InterpretableMultiHeadAttention on 8 Trainium2 NeuronCores.

Model (reference): qkv = x @ W_qkv; 16 q/k heads of 64, one shared v head;
causal softmax attention per head with shared V; mean over heads; @ W_out.

Sharding: core = (batch b, head-group hg of 8 heads). Each core computes its
8 heads' attention for its batch, applies (W_out/16) to the head-sum, and the
host adds the two head-group partials per batch.

Layout strategy (the TRN2 cost model charges matmuls per OUTPUT FREE ROW,
independent of K/M — so every matmul keeps its output free dim as small as
the math allows, and softmax probabilities are consumed as lhsT):
  qT/kT   [dh, t]  <- lhsT = W-slice [d, 128], rhs = xb [d, t]  (bf16, N=512)
  v       [t, dh]  <- lhsT = xb [d, t-tile],  rhs = wv [d, 64]  (bf16, N=64)
  scoresT [s, t]   <- lhsT = kT [dh, s-tile], rhs = qT [dh, w]  (bf16)
  expT    [s, t]   <- ONE exp activation per (head, s-row), w <= 1024; pairs
                      of narrow rows share one S tile (one per PSUM bank)
  causal diag mask <- Pool-engine affine_select zeroing expT below the diag
  pv      [t, dh]  <- lhsT = expT [s, t-tile], rhs = v [s, 64] (bf16, N=64:
                      2.3x less PE than the [dh, t] layout); softmax row-sums
                      accumulate beside it via N=1 ones-column matmuls
  normalize+head-sum on DVE: per-partition reciprocal + broadcast multiply
  out     [t, m]   <- lhsT = accT [dh, t-tile] (PE-transposed), rhs = W_out/16
Causality at 128-block granularity (136/256 blocks). The t-range is split in
two 1024-wide passes so each pair's pv/dn accumulators fit 3 PSUM banks (one
accumulation group per bank: a matmul `start` wipes its whole bank). The two
heads of a pair interleave row-by-row so PE fills the Act engine's exp
latency; the next pair's QKV projection trickles behind at lower priority.
No softmax max-subtraction: scores/8 ~ N(0,1) so exp is well-bounded.
"""

import numpy as np

import concourse.bass as bass
import concourse.mybir as mybir
import concourse.tile as tile
from concourse.bass_utils import run_bass_kernel_spmd
from concourse.masks import make_identity

F32 = mybir.dt.float32
BF = mybir.dt.bfloat16  # 1 cyc/row at any N

B, T, D = 4, 2048, 1024
H, DH = 16, 64
HPC = 8          # heads per core
NPAIR = HPC // 2
DCH = D // 128   # 8 contraction chunks
N_CORES = 8
_uid = [0]


def _split_multiwaits(nc, maxw=1):
    """walrus rejects instructions with multiple sync waits (observed on the
    Tile exit drain). Move extra waits onto same-engine NoOps just before."""
    for _name, bbh in nc.bb_map.items():
        bb = bbh.bb if hasattr(bbh, "bb") else bbh
        insts = bb.instructions
        new = []
        for inst in insts:
            si = inst.sync_info
            if si is not None and len(si.on_wait) > maxw:
                waits = list(si.on_wait)
                extra, keep = waits[:-maxw], waits[-maxw:]
                for k in range(0, len(extra), maxw):
                    _uid[0] += 1
                    nop = mybir.InstNoOp(
                        name=f"I-waitsplit-{_uid[0]}", ins=[], outs=[]
                    )
                    nop.engine = inst.engine
                    nop.sync_info = mybir.SyncInfo(
                        on_wait=extra[k : k + maxw], on_update=[]
                    )
                    new.append(nop)
                inst.sync_info = mybir.SyncInfo(
                    on_wait=keep, on_update=list(si.on_update)
                )
            new.append(inst)
        insts[:] = new


def _emit_body(nc, tc, xb, wqk, wv, wout, out):
    Exp = mybir.ActivationFunctionType.Exp
    ts = bass.ts

    from contextlib import ExitStack

    ctx = ExitStack()
    consts = ctx.enter_context(tc.tile_pool(name="consts", bufs=1))
    ident_f = consts.tile([128, 128], F32)
    make_identity(nc, ident_f)
    wout_sb = consts.tile([64, 1024], BF)
    nc.gpsimd.dma_start(out=wout_sb, in_=wout[:])
    v_sb = consts.tile([128, 16, 65], BF)   # v tiles [t,dh] + ones column
    nc.vector.memset(v_sb[:, :, 64:65], 1.0)
    qk_sb = consts.tile([128, 8, T], BF)    # groups: q-pairs 0..3, k-pairs 4..7
    acc = consts.tile([128, 16, 64], F32)   # sum_h attn_h/denom_h, [t, dh]
    nc.vector.memset(acc, 0.0)
    accT = consts.tile([64, 16, 128], BF)
    ot_pool = ctx.enter_context(tc.tile_pool(name="ot", bufs=10))

    sb_ctx = ExitStack()
    xt_pool = sb_ctx.enter_context(tc.tile_pool(name="xt", bufs=1, side="right"))
    w_pool = sb_ctx.enter_context(tc.tile_pool(name="w", bufs=1, side="right"))
    xb_sb = xt_pool.tile([128, DCH, T], BF)
    wqk_sb = w_pool.tile([128, DCH, 1024], BF)
    wv_sb = w_pool.tile([128, DCH, 64], BF)
    # wqk is packed pair-major on the host: pair m at cols [256m, 256m+256)
    # (q then k). Inputs arrive as a few large HWDGE transfers (Pool-issued
    # per-chunk DMAs pay ~1us software-DGE serialization each); pair 0's wqk
    # slice goes ahead of the rest so the first pair unblocks right after xb.
    def r3(ap, p=128):
        return ap.rearrange("(dc p) c -> p dc c", p=p)

    nc.scalar.dma_start(out=wv_sb, in_=r3(wv[:]))
    nc.scalar.dma_start(out=xb_sb[:, 0:4, :], in_=r3(xb[0:512, :]))
    nc.sync.dma_start(out=wqk_sb[:, :, 0:256], in_=r3(wqk[:, 0:256]))
    nc.scalar.dma_start(out=xb_sb[:, 4:8, :], in_=r3(xb[512:1024, :]))
    # The non-pair-0 weights must not cut ahead of xb on the DMA engines
    # (xb gates everything). A tiny Pool copy reading xb into the transfer's
    # destination region forces the DMA (WAW) to request only after xb lands.
    nc.gpsimd.tensor_copy(wqk_sb[:, 7:8, 256:257], xb_sb[:, 7:8, 2047:2048])
    nc.gpsimd.dma_start(out=wqk_sb[:, :, 256:1024], in_=r3(wqk[:, 256:1024]))

    qkctx = ExitStack()
    psqk = qkctx.enter_context(
        tc.tile_pool(name="psqk", bufs=1, space="PSUM", side="right")
    )

    qk0ctx = ExitStack()
    psqk0 = qk0ctx.enter_context(
        tc.tile_pool(name="psqk0", bufs=3, space="PSUM", side="right")
    )

    # ---- v projection: out[t, dh], all 16 t-tiles chained per dc.
    # One accumulation group per PSUM bank (a start wipes the whole bank), so
    # start only on the first touch of each bank, stop on the last.
    psv_ctx = ExitStack()
    psv = psv_ctx.enter_context(
        tc.tile_pool(name="psv", bufs=1, space="PSUM", side="right")
    )
    v_ps = psv.tile([128, 16, 64], F32)

    def emit_vproj(dcs):
        for dc in dcs:
            for tt in range(16):
                nc.tensor.matmul(
                    v_ps[:, tt, :],
                    xb_sb[:, dc, ts(tt, 128)],
                    wv_sb[:, dc, :],
                    start=(dc == 0 and tt % 8 == 0),
                    stop=(dc == DCH - 1 and tt % 8 == 7),
                )



    def emit_qkproj(m, pool=None, use_act=False, tccs=range(4)):
        # project q-pair m (group m) and k-pair m (group 4+m) -> qk_sb bf16.
        # q/k alternate per t-chunk so the first attention rows unblock early.
        # Pair 0 runs before any exp, so its copies can also use the idle Act
        # engine to halve the copy serialization on the critical path.
        for tcc in tccs:
            for g, off in ((m, 256 * m), (4 + m, 256 * m + 128)):
                ps = (pool or psqk).tile([128, 512], F32, tag="qk")
                for dc in range(DCH):
                    nc.tensor.matmul(
                        ps,
                        wqk_sb[:, dc, off : off + 128],
                        xb_sb[:, dc, ts(tcc, 512)],
                        start=(dc == 0),
                        stop=(dc == DCH - 1),
                    )
                if use_act:
                    nc.scalar.copy(qk_sb[:, g, ts(tcc, 512)], ps)
                else:
                    nc.vector.tensor_copy(qk_sb[:, g, ts(tcc, 512)], ps)

    emit_vproj(range(DCH))

    emit_qkproj(0, pool=psqk0, use_act=True)
    # v copy emitted after pair-0's q copies so DVE's in-order stream
    # doesn't head-of-line block the attention-critical copies behind it
    nc.vector.tensor_copy(v_sb[:, :, 0:64], v_ps)
    psv_ctx.close()
    qk0ctx.close()

    at_ctx = ExitStack()
    psS = at_ctx.enter_context(tc.tile_pool(name="psS", bufs=2, space="PSUM"))
    psPV = at_ctx.enter_context(tc.tile_pool(name="psPV", bufs=1, space="PSUM"))
    et_pool = at_ctx.enter_context(tc.tile_pool(name="et", bufs=8))
    nrm_pool = at_ctx.enter_context(tc.tile_pool(name="nrm", bufs=2))

    def emit_attn_pass(m, pass_):
        # The two heads of the pair interleave at row granularity: while the
        # Act engine exponentiates head A's row, PE runs head B's scores/PV.
        # pv holds both heads' accumulators (one bank per head = one
        # accumulation group each); dn shares a single bank as ONE group
        # spanning both heads (disjoint slices).
        if True:
            t0 = 1024 * pass_
            jb = 8 * pass_
            imax = 8 + 8 * pass_
            pv = psPV.tile([128, 2, 8, 64], F32, tag="pv")
            dn = psPV.tile([128, 2, 8], F32, tag="dn")

            def emit_pv_row(ets, i, t_start, off):
                # PV emission lags one row group behind S/exp: keeps PE's
                # in-order stream from head-of-line blocking on the pv slot
                # (only freed by the previous pass's normalize) or on exp(i).
                first = i == 0
                last = i == imax - 1
                for hh in (0, 1):
                    for jj in range(max(i, jb), jb + 8):
                        lo = off + 128 * jj - t_start
                        eti = ets[hh][:, lo : lo + 128]
                        st = first and jj == jb
                        nc.tensor.matmul(
                            pv[:, hh, jj - jb, :],
                            eti,
                            v_sb[:, i, 0:64],
                            start=st,
                            stop=last,
                        )
                        nc.tensor.matmul(
                            dn[:, hh, jj - jb : jj - jb + 1],
                            eti,
                            v_sb[:, i, 64:65],
                            start=(st and hh == 0),
                            stop=(last and hh == 1),
                        )

            def emit_srow(S, et_w, i, t_start, w, off, own, fill_to):
                # scores for row i into S[:, off:off+w]; each PSUM bank of the
                # tile holds exactly one accumulation group. `own` = this row
                # owns bank0 of the tile starting at off=0 (w may span both
                # banks); merged rows sit alone in bank1 (off=512, w<=512).
                lim = min(w, 512)
                p0 = 64 * own
                kTi = qk_sb[p0 : p0 + 64, 4 + m, ts(i, 128)]
                qT = qk_sb[p0 : p0 + 64, m, :]
                junk = fill_to - lim if off == 0 else 0
                nc.tensor.matmul(
                    S[:, off : off + lim],
                    kTi,
                    qT[:, t_start : t_start + lim],
                    start=True,
                    stop=(junk == 0),
                )
                if junk:
                    # merged tiles: pad bank0 to its full span with junk
                    # scores (same accumulation group; exp'd, never read)
                    nc.tensor.matmul(
                        S[:, off + lim : off + fill_to],
                        kTi,
                        qT[:, 0:junk],
                        start=False,
                        stop=True,
                    )
                if w > lim:
                    nc.tensor.matmul(
                        S[:, off + 512 : off + w],
                        kTi,
                        qT[:, t_start + 512 : t_start + w],
                        start=True,
                        stop=True,
                    )

            # rows with w <= 512 pair up two-per-S-tile (one per bank) so a
            # single exp instruction covers both; pairing (a,a+2) keeps the
            # wider row in bank0 with zero wasted exp work.
            if pass_ == 0:
                groups = [[0], [1], [2], [3], [4, 6], [5, 7]]
            else:
                groups = [[i] for i in range(12)] + [[12, 14], [13, 15]]
            pend = []
            for grp in groups:
                rows = []
                for r_idx, i in enumerate(grp):
                    t_start = max(128 * i, t0)
                    rows.append((i, t_start, t0 + 1024 - t_start, 512 * r_idx))
                ets = []
                for hh in (0, 1):
                    S = psS.tile([128, 1024], F32, tag="s")
                    et = et_pool.tile([128, 1024], BF, tag="et")
                    for i, t_start, w, off in rows:
                        ft = min(w, 512) if len(rows) == 1 else 512
                        emit_srow(S, et, i, t_start, w, off, hh, ft)
                    wtot = rows[-1][3] + rows[-1][2]
                    nc.scalar.activation(
                        et[:, 0:wtot], S[:, 0:wtot], Exp, scale=0.125
                    )
                    for i, t_start, w, off in rows:
                        if 128 * i >= t0:
                            # causal diag: zero et where col < row, on the
                            # otherwise-idle Pool engine instead of a PE
                            # additive-mask matmul
                            nc.gpsimd.affine_select(
                                out=et[:, off : off + 128],
                                in_=et[:, off : off + 128],
                                compare_op=mybir.AluOpType.is_ge,
                                fill=0.0,
                                base=0,
                                # keep (col - row) >= 0
                                pattern=[[1, 128]],
                                channel_multiplier=-1,
                            )
                    ets.append(et)
                new = [(ets, i, t_start, off) for i, t_start, w, off in rows]
                for p in pend:
                    emit_pv_row(*p)
                pend = new
            for p in pend:
                emit_pv_row(*p)
            # normalize by the row-sums; accumulate both heads into acc
            rcp = nrm_pool.tile([128, 2, 8], F32, tag="rcp")
            nc.vector.reciprocal(rcp, dn)
            prod = nrm_pool.tile([128, 2, 8, 64], F32, tag="prod")
            nc.vector.tensor_mul(
                prod, pv, rcp.unsqueeze(3).broadcast_to([128, 2, 8, 64])
            )
            sum2 = nrm_pool.tile([128, 8, 64], F32, tag="sum2")
            nc.vector.tensor_add(sum2, prod[:, 0], prod[:, 1])
            nc.vector.tensor_add(
                acc[:, jb : jb + 8, :], acc[:, jb : jb + 8, :], sum2
            )

    def emit_stageD_tt(tt, psD, act_copy):
        pT = psD.tile([64, 128], F32, tag="d")
        nc.tensor.transpose(pT, acc[:, tt, :], ident_f)
        nc.vector.tensor_copy(accT[:, tt, :], pT)
        ot = ot_pool.tile([128, 1024], BF, tag="ot")
        for mc in range(2):
            po = psD.tile([128, 512], F32, tag="d")
            nc.tensor.matmul(
                po,
                accT[:, tt, :],
                wout_sb[:, ts(mc, 512)],
                start=True,
                stop=True,
            )
            if mc == 1 and act_copy:
                nc.scalar.copy(ot[:, 512:1024], po)
            else:
                nc.vector.tensor_copy(ot[:, ts(mc, 512)], po)
        nc.sync.dma_start(out=out[ts(tt, 128), :], in_=ot)

    dA_ctx = ExitStack()
    for m in range(NPAIR):
        emit_attn_pass(m, 0)
        if m + 1 == NPAIR:
            # stage D for the first t-half trickles through the last pair's
            # pass B on the bank freed by psqk (single slot self-throttles).
            qkctx.close()
            sb_ctx.close()
            psD_a = dA_ctx.enter_context(
                tc.tile_pool(name="psDa", bufs=1, space="PSUM", side="right")
            )
        emit_attn_pass(m, 1)
        if m + 1 < NPAIR:
            emit_qkproj(m + 1)
        if m + 1 == NPAIR:
            for tt in range(8):
                emit_stageD_tt(tt, psD_a, act_copy=False)
    dA_ctx.close()
    at_ctx.close()

    # ---- stage D, second t-half: runs at the tail with roomy pools ----
    d_ctx = ExitStack()
    psD_b = d_ctx.enter_context(tc.tile_pool(name="psDb", bufs=8, space="PSUM"))
    for tt in range(8, 16):
        emit_stageD_tt(tt, psD_b, act_copy=True)
    d_ctx.close()
    ctx.close()


_NC_CACHE = [None]


def build_nc():
    if _NC_CACHE[0] is not None:
        return _NC_CACHE[0]
    nc = bass.Bass("TRN2", target_bir_lowering=False, debug=False)
    xb = nc.declare_dram_parameter("xb", [D, T], BF, isOutput=False)
    wqk = nc.declare_dram_parameter("wqk", [D, 1024], BF, isOutput=False)
    wv = nc.declare_dram_parameter("wv", [D, 64], BF, isOutput=False)
    wout = nc.declare_dram_parameter("wout", [64, D], BF, isOutput=False)
    out = nc.declare_dram_parameter("out", [T, D], BF, isOutput=True)
    with tile.TileContext(nc) as tc, nc.allow_low_precision(
        reason="f32r qkv proj + bf16 attention path; ~6e-3 rel err vs fp32 ref"
    ):
        _emit_body(nc, tc, xb, wqk, wv, wout, out)
    _split_multiwaits(nc, maxw=1)
    _NC_CACHE[0] = nc
    return nc


def make_in_maps(x, W_qkv, W_out):
    import ml_dtypes

    bf16 = ml_dtypes.bfloat16
    wv = np.ascontiguousarray(W_qkv[:, 2 * H * DH :]).astype(bf16)
    wout = np.ascontiguousarray(np.asarray(W_out) / float(H)).astype(bf16)
    in_maps = []
    for core in range(N_CORES):
        b, hg = core // 2, core % 2
        xTf = np.ascontiguousarray(np.asarray(x[b]).T, dtype=np.float32)
        cols = []
        for mp in range(NPAIR):  # pair-major: [q-pair | k-pair] per pair
            h0 = hg * HPC + 2 * mp
            for off in (0, H * DH):
                cols.append(W_qkv[:, off + h0 * DH : off + (h0 + 2) * DH])
        wqk = np.ascontiguousarray(np.concatenate(cols, axis=1)).astype(bf16)
        in_maps.append(
            {
                "xb": xTf.astype(bf16),
                "wqk": wqk,
                "wv": wv,
                "wout": wout,
            }
        )
    return in_maps


def kernel(x, W_qkv, W_out, _trace=False, _trace_kwargs=None):
    nc = build_nc()
    in_maps = make_in_maps(x, W_qkv, W_out)
    res = run_bass_kernel_spmd(
        nc, in_maps, list(range(N_CORES)), trace=_trace, **(_trace_kwargs or {})
    )
    out = np.empty((B, T, D), dtype=np.float32)
    for b in range(B):
        out[b] = np.asarray(res.results[2 * b]["out"], dtype=np.float32) + np.asarray(res.results[2 * b + 1]["out"], dtype=np.float32)
    if _trace:
        return out, res
    return out


# revision 64
# speedup vs baseline: 1.0032x; 1.0010x over previous
"""InterpretableMultiHeadAttention on 8 Trainium2 NeuronCores.

Model (reference): qkv = x @ W_qkv; 16 q/k heads of 64, one shared v head;
causal softmax attention per head with shared V; mean over heads; @ W_out.

Sharding: core = (batch b, head-group hg of 8 heads). Each core computes its
8 heads' attention for its batch, applies (W_out/16) to the head-sum, and the
host adds the two head-group partials per batch.

Layout strategy (the TRN2 cost model charges matmuls per OUTPUT FREE ROW,
independent of K/M — so every matmul keeps its output free dim as small as
the math allows, and softmax probabilities are consumed as lhsT):
  qT/kT   [dh, t]  <- lhsT = W-slice [d, 128], rhs = xb [d, t]  (bf16, N=512)
  v       [t, dh]  <- lhsT = xb [d, t-tile],  rhs = wv [d, 64]  (bf16, N=64)
  scoresT [s, t]   <- lhsT = kT [dh, s-tile], rhs = qT [dh, w]  (bf16)
  expT    [s, t]   <- ONE exp activation per (head, s-row), w <= 1024; pairs
                      of narrow rows share one S tile (one per PSUM bank)
  causal diag mask <- Pool-engine affine_select zeroing expT below the diag
  pv      [t, dh]  <- lhsT = expT [s, t-tile], rhs = v [s, 64] (bf16, N=64:
                      2.3x less PE than the [dh, t] layout); softmax row-sums
                      accumulate beside it via N=1 ones-column matmuls
  normalize+head-sum on DVE: per-partition reciprocal + broadcast multiply
  out     [t, m]   <- lhsT = accT [dh, t-tile] (PE-transposed), rhs = W_out/16
Causality at 128-block granularity (136/256 blocks). The t-range is split in
two 1024-wide passes so each pair's pv/dn accumulators fit 3 PSUM banks (one
accumulation group per bank: a matmul `start` wipes its whole bank). The two
heads of a pair interleave row-by-row so PE fills the Act engine's exp
latency; the next pair's QKV projection trickles behind at lower priority.
No softmax max-subtraction: scores/8 ~ N(0,1) so exp is well-bounded.
"""

import numpy as np

import concourse.bass as bass
import concourse.mybir as mybir
import concourse.tile as tile
from concourse.bass_utils import run_bass_kernel_spmd
from concourse.masks import make_identity

F32 = mybir.dt.float32
BF = mybir.dt.bfloat16  # 1 cyc/row at any N

B, T, D = 4, 2048, 1024
H, DH = 16, 64
HPC = 8          # heads per core
NPAIR = HPC // 2
DCH = D // 128   # 8 contraction chunks
N_CORES = 8
_uid = [0]


def _split_multiwaits(nc, maxw=1):
    """walrus rejects instructions with multiple sync waits (observed on the
    Tile exit drain). Move extra waits onto same-engine NoOps just before."""
    for _name, bbh in nc.bb_map.items():
        bb = bbh.bb if hasattr(bbh, "bb") else bbh
        insts = bb.instructions
        new = []
        for inst in insts:
            si = inst.sync_info
            if si is not None and len(si.on_wait) > maxw:
                waits = list(si.on_wait)
                extra, keep = waits[:-maxw], waits[-maxw:]
                for k in range(0, len(extra), maxw):
                    _uid[0] += 1
                    nop = mybir.InstNoOp(
                        name=f"I-waitsplit-{_uid[0]}", ins=[], outs=[]
                    )
                    nop.engine = inst.engine
                    nop.sync_info = mybir.SyncInfo(
                        on_wait=extra[k : k + maxw], on_update=[]
                    )
                    new.append(nop)
                inst.sync_info = mybir.SyncInfo(
                    on_wait=keep, on_update=list(si.on_update)
                )
            new.append(inst)
        insts[:] = new


def _emit_body(nc, tc, xb, wqk, wv, wout, out):
    Exp = mybir.ActivationFunctionType.Exp
    ts = bass.ts

    from contextlib import ExitStack

    ctx = ExitStack()
    consts = ctx.enter_context(tc.tile_pool(name="consts", bufs=1))
    ident_f = consts.tile([128, 128], F32)
    make_identity(nc, ident_f)
    wout_sb = consts.tile([64, 1024], BF)
    nc.gpsimd.dma_start(out=wout_sb, in_=wout[:])
    v_sb = consts.tile([128, 16, 65], BF)   # v tiles [t,dh] + ones column
    nc.vector.memset(v_sb[:, :, 64:65], 1.0)
    qk_sb = consts.tile([128, 8, T], BF)    # groups: q-pairs 0..3, k-pairs 4..7
    acc = consts.tile([128, 16, 64], F32)   # sum_h attn_h/denom_h, [t, dh]
    nc.vector.memset(acc, 0.0)
    accT = consts.tile([64, 16, 128], BF)
    ot_pool = ctx.enter_context(tc.tile_pool(name="ot", bufs=12))

    sb_ctx = ExitStack()
    xt_pool = sb_ctx.enter_context(tc.tile_pool(name="xt", bufs=1, side="right"))
    w_pool = sb_ctx.enter_context(tc.tile_pool(name="w", bufs=1, side="right"))
    xb_sb = xt_pool.tile([128, DCH, T], BF)
    wqk_sb = w_pool.tile([128, DCH, 1024], BF)
    wv_sb = w_pool.tile([128, DCH, 64], BF)
    # wqk is packed pair-major on the host: pair m at cols [256m, 256m+256)
    # (q then k). Inputs arrive as a few large HWDGE transfers (Pool-issued
    # per-chunk DMAs pay ~1us software-DGE serialization each); pair 0's wqk
    # slice goes ahead of the rest so the first pair unblocks right after xb.
    def r3(ap, p=128):
        return ap.rearrange("(dc p) c -> p dc c", p=p)

    nc.scalar.dma_start(out=wv_sb, in_=r3(wv[:]))
    nc.scalar.dma_start(out=xb_sb[:, 0:4, :], in_=r3(xb[0:512, :]))
    nc.sync.dma_start(out=wqk_sb[:, :, 0:256], in_=r3(wqk[:, 0:256]))
    nc.scalar.dma_start(out=xb_sb[:, 4:8, :], in_=r3(xb[512:1024, :]))
    # The non-pair-0 weights must not cut ahead of xb on the DMA engines
    # (xb gates everything). A tiny Pool copy reading xb into the transfer's
    # destination region forces the DMA (WAW) to request only after xb lands.
    nc.gpsimd.tensor_copy(wqk_sb[:, 7:8, 256:257], xb_sb[:, 7:8, 2047:2048])
    nc.gpsimd.dma_start(out=wqk_sb[:, :, 256:1024], in_=r3(wqk[:, 256:1024]))

    qkctx = ExitStack()
    psqk = qkctx.enter_context(
        tc.tile_pool(name="psqk", bufs=1, space="PSUM", side="right")
    )

    qk0ctx = ExitStack()
    psqk0 = qk0ctx.enter_context(
        tc.tile_pool(name="psqk0", bufs=3, space="PSUM", side="right")
    )

    # ---- v projection: out[t, dh], all 16 t-tiles chained per dc.
    # One accumulation group per PSUM bank (a start wipes the whole bank), so
    # start only on the first touch of each bank, stop on the last.
    psv_ctx = ExitStack()
    psv = psv_ctx.enter_context(
        tc.tile_pool(name="psv", bufs=1, space="PSUM", side="right")
    )
    v_ps = psv.tile([128, 16, 64], F32)

    def emit_vproj(dcs):
        for dc in dcs:
            for tt in range(16):
                nc.tensor.matmul(
                    v_ps[:, tt, :],
                    xb_sb[:, dc, ts(tt, 128)],
                    wv_sb[:, dc, :],
                    start=(dc == 0 and tt % 8 == 0),
                    stop=(dc == DCH - 1 and tt % 8 == 7),
                )



    def emit_qkproj(m, pool=None, use_act=False, tccs=range(4)):
        # project q-pair m (group m) and k-pair m (group 4+m) -> qk_sb bf16.
        # q/k alternate per t-chunk so the first attention rows unblock early.
        # Pair 0 runs before any exp, so its copies can also use the idle Act
        # engine to halve the copy serialization on the critical path.
        for tcc in tccs:
            for g, off in ((m, 256 * m), (4 + m, 256 * m + 128)):
                ps = (pool or psqk).tile([128, 512], F32, tag="qk")
                for dc in range(DCH):
                    nc.tensor.matmul(
                        ps,
                        wqk_sb[:, dc, off : off + 128],
                        xb_sb[:, dc, ts(tcc, 512)],
                        start=(dc == 0),
                        stop=(dc == DCH - 1),
                    )
                if use_act:
                    nc.scalar.copy(qk_sb[:, g, ts(tcc, 512)], ps)
                else:
                    nc.vector.tensor_copy(qk_sb[:, g, ts(tcc, 512)], ps)

    emit_vproj(range(DCH))

    emit_qkproj(0, pool=psqk0, use_act=True)
    # v copy emitted after pair-0's q copies so DVE's in-order stream
    # doesn't head-of-line block the attention-critical copies behind it
    nc.vector.tensor_copy(v_sb[:, :, 0:64], v_ps)
    psv_ctx.close()
    qk0ctx.close()

    at_ctx = ExitStack()
    psS = at_ctx.enter_context(tc.tile_pool(name="psS", bufs=2, space="PSUM"))
    psPV = at_ctx.enter_context(tc.tile_pool(name="psPV", bufs=1, space="PSUM"))
    et_pool = at_ctx.enter_context(tc.tile_pool(name="et", bufs=8))
    nrm_pool = at_ctx.enter_context(tc.tile_pool(name="nrm", bufs=2))

    def emit_attn_pass(m, pass_):
        # The two heads of the pair interleave at row granularity: while the
        # Act engine exponentiates head A's row, PE runs head B's scores/PV.
        # pv holds both heads' accumulators (one bank per head = one
        # accumulation group each); dn shares a single bank as ONE group
        # spanning both heads (disjoint slices).
        if True:
            t0 = 1024 * pass_
            jb = 8 * pass_
            imax = 8 + 8 * pass_
            pv = psPV.tile([128, 2, 8, 64], F32, tag="pv")
            dn = psPV.tile([128, 2, 8], F32, tag="dn")

            def emit_pv_row(ets, i, t_start, off):
                # PV emission lags one row group behind S/exp: keeps PE's
                # in-order stream from head-of-line blocking on the pv slot
                # (only freed by the previous pass's normalize) or on exp(i).
                first = i == 0
                last = i == imax - 1
                for hh in (0, 1):
                    for jj in range(max(i, jb), jb + 8):
                        lo = off + 128 * jj - t_start
                        eti = ets[hh][:, lo : lo + 128]
                        st = first and jj == jb
                        nc.tensor.matmul(
                            pv[:, hh, jj - jb, :],
                            eti,
                            v_sb[:, i, 0:64],
                            start=st,
                            stop=last,
                        )
                        nc.tensor.matmul(
                            dn[:, hh, jj - jb : jj - jb + 1],
                            eti,
                            v_sb[:, i, 64:65],
                            start=(st and hh == 0),
                            stop=(last and hh == 1),
                        )

            def emit_srow(S, et_w, i, t_start, w, off, own, fill_to):
                # scores for row i into S[:, off:off+w]; each PSUM bank of the
                # tile holds exactly one accumulation group. `own` = this row
                # owns bank0 of the tile starting at off=0 (w may span both
                # banks); merged rows sit alone in bank1 (off=512, w<=512).
                lim = min(w, 512)
                p0 = 64 * own
                kTi = qk_sb[p0 : p0 + 64, 4 + m, ts(i, 128)]
                qT = qk_sb[p0 : p0 + 64, m, :]
                junk = fill_to - lim if off == 0 else 0
                nc.tensor.matmul(
                    S[:, off : off + lim],
                    kTi,
                    qT[:, t_start : t_start + lim],
                    start=True,
                    stop=(junk == 0),
                )
                if junk:
                    # merged tiles: pad bank0 to its full span with junk
                    # scores (same accumulation group; exp'd, never read)
                    nc.tensor.matmul(
                        S[:, off + lim : off + fill_to],
                        kTi,
                        qT[:, 0:junk],
                        start=False,
                        stop=True,
                    )
                if w > lim:
                    nc.tensor.matmul(
                        S[:, off + 512 : off + w],
                        kTi,
                        qT[:, t_start + 512 : t_start + w],
                        start=True,
                        stop=True,
                    )

            # rows with w <= 512 pair up two-per-S-tile (one per bank) so a
            # single exp instruction covers both; pairing (a,a+2) keeps the
            # wider row in bank0 with zero wasted exp work.
            if pass_ == 0:
                groups = [[0], [1], [2], [3], [4, 6], [5, 7]]
            else:
                groups = [[i] for i in range(12)] + [[12, 14], [13, 15]]
            pend = []
            for grp in groups:
                rows = []
                for r_idx, i in enumerate(grp):
                    t_start = max(128 * i, t0)
                    rows.append((i, t_start, t0 + 1024 - t_start, 512 * r_idx))
                ets = []
                for hh in (0, 1):
                    S = psS.tile([128, 1024], F32, tag="s")
                    et = et_pool.tile([128, 1024], BF, tag="et")
                    for i, t_start, w, off in rows:
                        ft = min(w, 512) if len(rows) == 1 else 512
                        emit_srow(S, et, i, t_start, w, off, hh, ft)
                    wtot = rows[-1][3] + rows[-1][2]
                    nc.scalar.activation(
                        et[:, 0:wtot], S[:, 0:wtot], Exp, scale=0.125
                    )
                    for i, t_start, w, off in rows:
                        if 128 * i >= t0:
                            # causal diag: zero et where col < row, on the
                            # otherwise-idle Pool engine instead of a PE
                            # additive-mask matmul
                            nc.gpsimd.affine_select(
                                out=et[:, off : off + 128],
                                in_=et[:, off : off + 128],
                                compare_op=mybir.AluOpType.is_ge,
                                fill=0.0,
                                base=0,
                                # keep (col - row) >= 0
                                pattern=[[1, 128]],
                                channel_multiplier=-1,
                            )
                    ets.append(et)
                new = [(ets, i, t_start, off) for i, t_start, w, off in rows]
                for p in pend:
                    emit_pv_row(*p)
                pend = new
            for p in pend:
                emit_pv_row(*p)
            # normalize by the row-sums; accumulate both heads into acc
            rcp = nrm_pool.tile([128, 2, 8], F32, tag="rcp")
            nc.vector.reciprocal(rcp, dn)
            prod = nrm_pool.tile([128, 2, 8, 64], F32, tag="prod")
            nc.vector.tensor_mul(
                prod, pv, rcp.unsqueeze(3).broadcast_to([128, 2, 8, 64])
            )
            sum2 = nrm_pool.tile([128, 8, 64], F32, tag="sum2")
            nc.vector.tensor_add(sum2, prod[:, 0], prod[:, 1])
            nc.vector.tensor_add(
                acc[:, jb : jb + 8, :], acc[:, jb : jb + 8, :], sum2
            )

    def emit_stageD_tt(tt, psD, act_copy):
        pT = psD.tile([64, 128], F32, tag="d")
        nc.tensor.transpose(pT, acc[:, tt, :], ident_f)
        nc.vector.tensor_copy(accT[:, tt, :], pT)
        ot = ot_pool.tile([128, 1024], BF, tag="ot")
        for mc in range(2):
            po = psD.tile([128, 512], F32, tag="d")
            nc.tensor.matmul(
                po,
                accT[:, tt, :],
                wout_sb[:, ts(mc, 512)],
                start=True,
                stop=True,
            )
            if mc == 1 and act_copy:
                nc.scalar.copy(ot[:, 512:1024], po)
            else:
                nc.vector.tensor_copy(ot[:, ts(mc, 512)], po)
        nc.sync.dma_start(out=out[ts(tt, 128), :], in_=ot)

    dA_ctx = ExitStack()
    for m in range(NPAIR):
        emit_attn_pass(m, 0)
        if m + 1 == NPAIR:
            # stage D for the first t-half trickles through the last pair's
            # pass B on the bank freed by psqk (single slot self-throttles).
            qkctx.close()
            sb_ctx.close()
            psD_a = dA_ctx.enter_context(
                tc.tile_pool(name="psDa", bufs=1, space="PSUM", side="right")
            )
        emit_attn_pass(m, 1)
        if m + 1 < NPAIR:
            emit_qkproj(m + 1)
        if m + 1 == NPAIR:
            for tt in range(8):
                emit_stageD_tt(tt, psD_a, act_copy=False)
    dA_ctx.close()
    at_ctx.close()

    # ---- stage D, second t-half: runs at the tail with roomy pools ----
    d_ctx = ExitStack()
    psD_b = d_ctx.enter_context(tc.tile_pool(name="psDb", bufs=8, space="PSUM"))
    for tt in range(8, 16):
        emit_stageD_tt(tt, psD_b, act_copy=True)
    d_ctx.close()
    ctx.close()


_NC_CACHE = [None]


def build_nc():
    if _NC_CACHE[0] is not None:
        return _NC_CACHE[0]
    nc = bass.Bass("TRN2", target_bir_lowering=False, debug=False)
    xb = nc.declare_dram_parameter("xb", [D, T], BF, isOutput=False)
    wqk = nc.declare_dram_parameter("wqk", [D, 1024], BF, isOutput=False)
    wv = nc.declare_dram_parameter("wv", [D, 64], BF, isOutput=False)
    wout = nc.declare_dram_parameter("wout", [64, D], BF, isOutput=False)
    out = nc.declare_dram_parameter("out", [T, D], BF, isOutput=True)
    with tile.TileContext(nc) as tc, nc.allow_low_precision(
        reason="f32r qkv proj + bf16 attention path; ~6e-3 rel err vs fp32 ref"
    ):
        _emit_body(nc, tc, xb, wqk, wv, wout, out)
    _split_multiwaits(nc, maxw=1)
    _NC_CACHE[0] = nc
    return nc


def make_in_maps(x, W_qkv, W_out):
    import ml_dtypes

    bf16 = ml_dtypes.bfloat16
    wv = np.ascontiguousarray(W_qkv[:, 2 * H * DH :]).astype(bf16)
    wout = np.ascontiguousarray(np.asarray(W_out) / float(H)).astype(bf16)
    in_maps = []
    for core in range(N_CORES):
        b, hg = core // 2, core % 2
        xTf = np.ascontiguousarray(np.asarray(x[b]).T, dtype=np.float32)
        cols = []
        for mp in range(NPAIR):  # pair-major: [q-pair | k-pair] per pair
            h0 = hg * HPC + 2 * mp
            for off in (0, H * DH):
                cols.append(W_qkv[:, off + h0 * DH : off + (h0 + 2) * DH])
        wqk = np.ascontiguousarray(np.concatenate(cols, axis=1)).astype(bf16)
        in_maps.append(
            {
                "xb": xTf.astype(bf16),
                "wqk": wqk,
                "wv": wv,
                "wout": wout,
            }
        )
    return in_maps


def kernel(x, W_qkv, W_out, _trace=False, _trace_kwargs=None):
    nc = build_nc()
    in_maps = make_in_maps(x, W_qkv, W_out)
    res = run_bass_kernel_spmd(
        nc, in_maps, list(range(N_CORES)), trace=_trace, **(_trace_kwargs or {})
    )
    out = np.empty((B, T, D), dtype=np.float32)
    for b in range(B):
        out[b] = np.asarray(res.results[2 * b]["out"], dtype=np.float32) + np.asarray(res.results[2 * b + 1]["out"], dtype=np.float32)
    if _trace:
        return out, res
    return out


# revision 65
# speedup vs baseline: 1.0042x; 1.0010x over previous
"""InterpretableMultiHeadAttention on 8 Trainium2 NeuronCores.

Model (reference): qkv = x @ W_qkv; 16 q/k heads of 64, one shared v head;
causal softmax attention per head with shared V; mean over heads; @ W_out.

Sharding: core = (batch b, head-group hg of 8 heads). Each core computes its
8 heads' attention for its batch, applies (W_out/16) to the head-sum, and the
host adds the two head-group partials per batch.

Layout strategy (the TRN2 cost model charges matmuls per OUTPUT FREE ROW,
independent of K/M — so every matmul keeps its output free dim as small as
the math allows, and softmax probabilities are consumed as lhsT):
  qT/kT   [dh, t]  <- lhsT = W-slice [d, 128], rhs = xb [d, t]  (bf16, N=512)
  v       [t, dh]  <- lhsT = xb [d, t-tile],  rhs = wv [d, 64]  (bf16, N=64)
  scoresT [s, t]   <- lhsT = kT [dh, s-tile], rhs = qT [dh, w]  (bf16)
  expT    [s, t]   <- ONE exp activation per (head, s-row), w <= 1024; pairs
                      of narrow rows share one S tile (one per PSUM bank)
  causal diag mask <- Pool-engine affine_select zeroing expT below the diag
  pv      [t, dh]  <- lhsT = expT [s, t-tile], rhs = v [s, 64] (bf16, N=64:
                      2.3x less PE than the [dh, t] layout); softmax row-sums
                      accumulate beside it via N=1 ones-column matmuls
  normalize+head-sum on DVE: per-partition reciprocal + broadcast multiply
  out     [t, m]   <- lhsT = accT [dh, t-tile] (PE-transposed), rhs = W_out/16
Causality at 128-block granularity (136/256 blocks). The t-range is split in
two 1024-wide passes so each pair's pv/dn accumulators fit 3 PSUM banks (one
accumulation group per bank: a matmul `start` wipes its whole bank). The two
heads of a pair interleave row-by-row so PE fills the Act engine's exp
latency; the next pair's QKV projection trickles behind at lower priority.
No softmax max-subtraction: scores/8 ~ N(0,1) so exp is well-bounded.
"""

import numpy as np

import concourse.bass as bass
import concourse.mybir as mybir
import concourse.tile as tile
from concourse.bass_utils import run_bass_kernel_spmd
from concourse.masks import make_identity

F32 = mybir.dt.float32
BF = mybir.dt.bfloat16  # 1 cyc/row at any N

B, T, D = 4, 2048, 1024
H, DH = 16, 64
HPC = 8          # heads per core
NPAIR = HPC // 2
DCH = D // 128   # 8 contraction chunks
N_CORES = 8
_uid = [0]


def _split_multiwaits(nc, maxw=1):
    """walrus rejects instructions with multiple sync waits (observed on the
    Tile exit drain). Move extra waits onto same-engine NoOps just before."""
    for _name, bbh in nc.bb_map.items():
        bb = bbh.bb if hasattr(bbh, "bb") else bbh
        insts = bb.instructions
        new = []
        for inst in insts:
            si = inst.sync_info
            if si is not None and len(si.on_wait) > maxw:
                waits = list(si.on_wait)
                extra, keep = waits[:-maxw], waits[-maxw:]
                for k in range(0, len(extra), maxw):
                    _uid[0] += 1
                    nop = mybir.InstNoOp(
                        name=f"I-waitsplit-{_uid[0]}", ins=[], outs=[]
                    )
                    nop.engine = inst.engine
                    nop.sync_info = mybir.SyncInfo(
                        on_wait=extra[k : k + maxw], on_update=[]
                    )
                    new.append(nop)
                inst.sync_info = mybir.SyncInfo(
                    on_wait=keep, on_update=list(si.on_update)
                )
            new.append(inst)
        insts[:] = new


def _emit_body(nc, tc, xb, wqk, wv, wout, out):
    Exp = mybir.ActivationFunctionType.Exp
    ts = bass.ts

    from contextlib import ExitStack

    ctx = ExitStack()
    consts = ctx.enter_context(tc.tile_pool(name="consts", bufs=1))
    ident_f = consts.tile([128, 128], F32)
    make_identity(nc, ident_f)
    wout_sb = consts.tile([64, 1024], BF)
    nc.gpsimd.dma_start(out=wout_sb, in_=wout[:])
    v_sb = consts.tile([128, 16, 65], BF)   # v tiles [t,dh] + ones column
    nc.vector.memset(v_sb[:, :, 64:65], 1.0)
    qk_sb = consts.tile([128, 8, T], BF)    # groups: q-pairs 0..3, k-pairs 4..7
    acc = consts.tile([128, 16, 64], F32)   # sum_h attn_h/denom_h, [t, dh]
    nc.vector.memset(acc, 0.0)
    accT = consts.tile([64, 16, 128], BF)
    ot_pool = ctx.enter_context(tc.tile_pool(name="ot", bufs=14))

    sb_ctx = ExitStack()
    xt_pool = sb_ctx.enter_context(tc.tile_pool(name="xt", bufs=1, side="right"))
    w_pool = sb_ctx.enter_context(tc.tile_pool(name="w", bufs=1, side="right"))
    xb_sb = xt_pool.tile([128, DCH, T], BF)
    wqk_sb = w_pool.tile([128, DCH, 1024], BF)
    wv_sb = w_pool.tile([128, DCH, 64], BF)
    # wqk is packed pair-major on the host: pair m at cols [256m, 256m+256)
    # (q then k). Inputs arrive as a few large HWDGE transfers (Pool-issued
    # per-chunk DMAs pay ~1us software-DGE serialization each); pair 0's wqk
    # slice goes ahead of the rest so the first pair unblocks right after xb.
    def r3(ap, p=128):
        return ap.rearrange("(dc p) c -> p dc c", p=p)

    nc.scalar.dma_start(out=wv_sb, in_=r3(wv[:]))
    nc.scalar.dma_start(out=xb_sb[:, 0:4, :], in_=r3(xb[0:512, :]))
    nc.sync.dma_start(out=wqk_sb[:, :, 0:256], in_=r3(wqk[:, 0:256]))
    nc.scalar.dma_start(out=xb_sb[:, 4:8, :], in_=r3(xb[512:1024, :]))
    # The non-pair-0 weights must not cut ahead of xb on the DMA engines
    # (xb gates everything). A tiny Pool copy reading xb into the transfer's
    # destination region forces the DMA (WAW) to request only after xb lands.
    nc.gpsimd.tensor_copy(wqk_sb[:, 7:8, 256:257], xb_sb[:, 7:8, 2047:2048])
    nc.gpsimd.dma_start(out=wqk_sb[:, :, 256:1024], in_=r3(wqk[:, 256:1024]))

    qkctx = ExitStack()
    psqk = qkctx.enter_context(
        tc.tile_pool(name="psqk", bufs=1, space="PSUM", side="right")
    )

    qk0ctx = ExitStack()
    psqk0 = qk0ctx.enter_context(
        tc.tile_pool(name="psqk0", bufs=3, space="PSUM", side="right")
    )

    # ---- v projection: out[t, dh], all 16 t-tiles chained per dc.
    # One accumulation group per PSUM bank (a start wipes the whole bank), so
    # start only on the first touch of each bank, stop on the last.
    psv_ctx = ExitStack()
    psv = psv_ctx.enter_context(
        tc.tile_pool(name="psv", bufs=1, space="PSUM", side="right")
    )
    v_ps = psv.tile([128, 16, 64], F32)

    def emit_vproj(dcs):
        for dc in dcs:
            for tt in range(16):
                nc.tensor.matmul(
                    v_ps[:, tt, :],
                    xb_sb[:, dc, ts(tt, 128)],
                    wv_sb[:, dc, :],
                    start=(dc == 0 and tt % 8 == 0),
                    stop=(dc == DCH - 1 and tt % 8 == 7),
                )



    def emit_qkproj(m, pool=None, use_act=False, tccs=range(4)):
        # project q-pair m (group m) and k-pair m (group 4+m) -> qk_sb bf16.
        # q/k alternate per t-chunk so the first attention rows unblock early.
        # Pair 0 runs before any exp, so its copies can also use the idle Act
        # engine to halve the copy serialization on the critical path.
        for tcc in tccs:
            for g, off in ((m, 256 * m), (4 + m, 256 * m + 128)):
                ps = (pool or psqk).tile([128, 512], F32, tag="qk")
                for dc in range(DCH):
                    nc.tensor.matmul(
                        ps,
                        wqk_sb[:, dc, off : off + 128],
                        xb_sb[:, dc, ts(tcc, 512)],
                        start=(dc == 0),
                        stop=(dc == DCH - 1),
                    )
                if use_act:
                    nc.scalar.copy(qk_sb[:, g, ts(tcc, 512)], ps)
                else:
                    nc.vector.tensor_copy(qk_sb[:, g, ts(tcc, 512)], ps)

    emit_vproj(range(DCH))

    emit_qkproj(0, pool=psqk0, use_act=True)
    # v copy emitted after pair-0's q copies so DVE's in-order stream
    # doesn't head-of-line block the attention-critical copies behind it
    nc.vector.tensor_copy(v_sb[:, :, 0:64], v_ps)
    psv_ctx.close()
    qk0ctx.close()

    at_ctx = ExitStack()
    psS = at_ctx.enter_context(tc.tile_pool(name="psS", bufs=2, space="PSUM"))
    psPV = at_ctx.enter_context(tc.tile_pool(name="psPV", bufs=1, space="PSUM"))
    et_pool = at_ctx.enter_context(tc.tile_pool(name="et", bufs=8))
    nrm_pool = at_ctx.enter_context(tc.tile_pool(name="nrm", bufs=2))

    def emit_attn_pass(m, pass_):
        # The two heads of the pair interleave at row granularity: while the
        # Act engine exponentiates head A's row, PE runs head B's scores/PV.
        # pv holds both heads' accumulators (one bank per head = one
        # accumulation group each); dn shares a single bank as ONE group
        # spanning both heads (disjoint slices).
        if True:
            t0 = 1024 * pass_
            jb = 8 * pass_
            imax = 8 + 8 * pass_
            pv = psPV.tile([128, 2, 8, 64], F32, tag="pv")
            dn = psPV.tile([128, 2, 8], F32, tag="dn")

            def emit_pv_row(ets, i, t_start, off):
                # PV emission lags one row group behind S/exp: keeps PE's
                # in-order stream from head-of-line blocking on the pv slot
                # (only freed by the previous pass's normalize) or on exp(i).
                first = i == 0
                last = i == imax - 1
                for hh in (0, 1):
                    for jj in range(max(i, jb), jb + 8):
                        lo = off + 128 * jj - t_start
                        eti = ets[hh][:, lo : lo + 128]
                        st = first and jj == jb
                        nc.tensor.matmul(
                            pv[:, hh, jj - jb, :],
                            eti,
                            v_sb[:, i, 0:64],
                            start=st,
                            stop=last,
                        )
                        nc.tensor.matmul(
                            dn[:, hh, jj - jb : jj - jb + 1],
                            eti,
                            v_sb[:, i, 64:65],
                            start=(st and hh == 0),
                            stop=(last and hh == 1),
                        )

            def emit_srow(S, et_w, i, t_start, w, off, own, fill_to):
                # scores for row i into S[:, off:off+w]; each PSUM bank of the
                # tile holds exactly one accumulation group. `own` = this row
                # owns bank0 of the tile starting at off=0 (w may span both
                # banks); merged rows sit alone in bank1 (off=512, w<=512).
                lim = min(w, 512)
                p0 = 64 * own
                kTi = qk_sb[p0 : p0 + 64, 4 + m, ts(i, 128)]
                qT = qk_sb[p0 : p0 + 64, m, :]
                junk = fill_to - lim if off == 0 else 0
                nc.tensor.matmul(
                    S[:, off : off + lim],
                    kTi,
                    qT[:, t_start : t_start + lim],
                    start=True,
                    stop=(junk == 0),
                )
                if junk:
                    # merged tiles: pad bank0 to its full span with junk
                    # scores (same accumulation group; exp'd, never read)
                    nc.tensor.matmul(
                        S[:, off + lim : off + fill_to],
                        kTi,
                        qT[:, 0:junk],
                        start=False,
                        stop=True,
                    )
                if w > lim:
                    nc.tensor.matmul(
                        S[:, off + 512 : off + w],
                        kTi,
                        qT[:, t_start + 512 : t_start + w],
                        start=True,
                        stop=True,
                    )

            # rows with w <= 512 pair up two-per-S-tile (one per bank) so a
            # single exp instruction covers both; pairing (a,a+2) keeps the
            # wider row in bank0 with zero wasted exp work.
            if pass_ == 0:
                groups = [[0], [1], [2], [3], [4, 6], [5, 7]]
            else:
                groups = [[i] for i in range(12)] + [[12, 14], [13, 15]]
            pend = []
            for grp in groups:
                rows = []
                for r_idx, i in enumerate(grp):
                    t_start = max(128 * i, t0)
                    rows.append((i, t_start, t0 + 1024 - t_start, 512 * r_idx))
                ets = []
                for hh in (0, 1):
                    S = psS.tile([128, 1024], F32, tag="s")
                    et = et_pool.tile([128, 1024], BF, tag="et")
                    for i, t_start, w, off in rows:
                        ft = min(w, 512) if len(rows) == 1 else 512
                        emit_srow(S, et, i, t_start, w, off, hh, ft)
                    wtot = rows[-1][3] + rows[-1][2]
                    nc.scalar.activation(
                        et[:, 0:wtot], S[:, 0:wtot], Exp, scale=0.125
                    )
                    for i, t_start, w, off in rows:
                        if 128 * i >= t0:
                            # causal diag: zero et where col < row, on the
                            # otherwise-idle Pool engine instead of a PE
                            # additive-mask matmul
                            nc.gpsimd.affine_select(
                                out=et[:, off : off + 128],
                                in_=et[:, off : off + 128],
                                compare_op=mybir.AluOpType.is_ge,
                                fill=0.0,
                                base=0,
                                # keep (col - row) >= 0
                                pattern=[[1, 128]],
                                channel_multiplier=-1,
                            )
                    ets.append(et)
                new = [(ets, i, t_start, off) for i, t_start, w, off in rows]
                for p in pend:
                    emit_pv_row(*p)
                pend = new
            for p in pend:
                emit_pv_row(*p)
            # normalize by the row-sums; accumulate both heads into acc
            rcp = nrm_pool.tile([128, 2, 8], F32, tag="rcp")
            nc.vector.reciprocal(rcp, dn)
            prod = nrm_pool.tile([128, 2, 8, 64], F32, tag="prod")
            nc.vector.tensor_mul(
                prod, pv, rcp.unsqueeze(3).broadcast_to([128, 2, 8, 64])
            )
            sum2 = nrm_pool.tile([128, 8, 64], F32, tag="sum2")
            nc.vector.tensor_add(sum2, prod[:, 0], prod[:, 1])
            nc.vector.tensor_add(
                acc[:, jb : jb + 8, :], acc[:, jb : jb + 8, :], sum2
            )

    def emit_stageD_tt(tt, psD, act_copy):
        pT = psD.tile([64, 128], F32, tag="d")
        nc.tensor.transpose(pT, acc[:, tt, :], ident_f)
        nc.vector.tensor_copy(accT[:, tt, :], pT)
        ot = ot_pool.tile([128, 1024], BF, tag="ot")
        for mc in range(2):
            po = psD.tile([128, 512], F32, tag="d")
            nc.tensor.matmul(
                po,
                accT[:, tt, :],
                wout_sb[:, ts(mc, 512)],
                start=True,
                stop=True,
            )
            if mc == 1 and act_copy:
                nc.scalar.copy(ot[:, 512:1024], po)
            else:
                nc.vector.tensor_copy(ot[:, ts(mc, 512)], po)
        nc.sync.dma_start(out=out[ts(tt, 128), :], in_=ot)

    dA_ctx = ExitStack()
    for m in range(NPAIR):
        emit_attn_pass(m, 0)
        if m + 1 == NPAIR:
            # stage D for the first t-half trickles through the last pair's
            # pass B on the bank freed by psqk (single slot self-throttles).
            qkctx.close()
            sb_ctx.close()
            psD_a = dA_ctx.enter_context(
                tc.tile_pool(name="psDa", bufs=1, space="PSUM", side="right")
            )
        emit_attn_pass(m, 1)
        if m + 1 < NPAIR:
            emit_qkproj(m + 1)
        if m + 1 == NPAIR:
            for tt in range(8):
                emit_stageD_tt(tt, psD_a, act_copy=False)
    dA_ctx.close()
    at_ctx.close()

    # ---- stage D, second t-half: runs at the tail with roomy pools ----
    d_ctx = ExitStack()
    psD_b = d_ctx.enter_context(tc.tile_pool(name="psDb", bufs=8, space="PSUM"))
    for tt in range(8, 16):
        emit_stageD_tt(tt, psD_b, act_copy=True)
    d_ctx.close()
    ctx.close()


_NC_CACHE = [None]


def build_nc():
    if _NC_CACHE[0] is not None:
        return _NC_CACHE[0]
    nc = bass.Bass("TRN2", target_bir_lowering=False, debug=False)
    xb = nc.declare_dram_parameter("xb", [D, T], BF, isOutput=False)
    wqk = nc.declare_dram_parameter("wqk", [D, 1024], BF, isOutput=False)
    wv = nc.declare_dram_parameter("wv", [D, 64], BF, isOutput=False)
    wout = nc.declare_dram_parameter("wout", [64, D], BF, isOutput=False)
    out = nc.declare_dram_parameter("out", [T, D], BF, isOutput=True)
    with tile.TileContext(nc) as tc, nc.allow_low_precision(
        reason="f32r qkv proj + bf16 attention path; ~6e-3 rel err vs fp32 ref"
    ):
        _emit_body(nc, tc, xb, wqk, wv, wout, out)
    _split_multiwaits(nc, maxw=1)
    _NC_CACHE[0] = nc
    return nc


def make_in_maps(x, W_qkv, W_out):
    import ml_dtypes

    bf16 = ml_dtypes.bfloat16
    wv = np.ascontiguousarray(W_qkv[:, 2 * H * DH :]).astype(bf16)
    wout = np.ascontiguousarray(np.asarray(W_out) / float(H)).astype(bf16)
    in_maps = []
    for core in range(N_CORES):
        b, hg = core // 2, core % 2
        xTf = np.ascontiguousarray(np.asarray(x[b]).T, dtype=np.float32)
        cols = []
        for mp in range(NPAIR):  # pair-major: [q-pair | k-pair] per pair
            h0 = hg * HPC + 2 * mp
            for off in (0, H * DH):
                cols.append(W_qkv[:, off + h0 * DH : off + (h0 + 2) * DH])
        wqk = np.ascontiguousarray(np.concatenate(cols, axis=1)).astype(bf16)
        in_maps.append(
            {
                "xb": xTf.astype(bf16),
                "wqk": wqk,
                "wv": wv,
                "wout": wout,
            }
        )
    return in_maps


def kernel(x, W_qkv, W_out, _trace=False, _trace_kwargs=None):
    nc = build_nc()
    in_maps = make_in_maps(x, W_qkv, W_out)
    res = run_bass_kernel_spmd(
        nc, in_maps, list(range(N_CORES)), trace=_trace, **(_trace_kwargs or {})
    )
    out = np.empty((B, T, D), dtype=np.float32)
    for b in range(B):
        out[b] = np.asarray(res.results[2 * b]["out"], dtype=np.float32) + np.asarray(res.results[2 * b + 1]["out"], dtype=np.float32)
    if _trace:
        return out, res
    return out


# revision 66
# speedup vs baseline: 1.0051x; 1.0009x over previous
"""InterpretableMultiHeadAttention on 8 Trainium2 NeuronCores.

Model (reference): qkv = x @ W_qkv; 16 q/k heads of 64, one shared v head;
causal softmax attention per head with shared V; mean over heads; @ W_out.

Sharding: core = (batch b, head-group hg of 8 heads). Each core computes its
8 heads' attention for its batch, applies (W_out/16) to the head-sum, and the
host adds the two head-group partials per batch.

Layout strategy (the TRN2 cost model charges matmuls per OUTPUT FREE ROW,
independent of K/M — so every matmul keeps its output free dim as small as
the math allows, and softmax probabilities are consumed as lhsT):
  qT/kT   [dh, t]  <- lhsT = W-slice [d, 128], rhs = xb [d, t]  (bf16, N=512)
  v       [t, dh]  <- lhsT = xb [d, t-tile],  rhs = wv [d, 64]  (bf16, N=64)
  scoresT [s, t]   <- lhsT = kT [dh, s-tile], rhs = qT [dh, w]  (bf16)
  expT    [s, t]   <- ONE exp activation per (head, s-row), w <= 1024; pairs
                      of narrow rows share one S tile (one per PSUM bank)
  causal diag mask <- Pool-engine affine_select zeroing expT below the diag
  pv      [t, dh]  <- lhsT = expT [s, t-tile], rhs = v [s, 64] (bf16, N=64:
                      2.3x less PE than the [dh, t] layout); softmax row-sums
                      accumulate beside it via N=1 ones-column matmuls
  normalize+head-sum on DVE: per-partition reciprocal + broadcast multiply
  out     [t, m]   <- lhsT = accT [dh, t-tile] (PE-transposed), rhs = W_out/16
Causality at 128-block granularity (136/256 blocks). The t-range is split in
two 1024-wide passes so each pair's pv/dn accumulators fit 3 PSUM banks (one
accumulation group per bank: a matmul `start` wipes its whole bank). The two
heads of a pair interleave row-by-row so PE fills the Act engine's exp
latency; the next pair's QKV projection trickles behind at lower priority.
No softmax max-subtraction: scores/8 ~ N(0,1) so exp is well-bounded.
"""

import numpy as np

import concourse.bass as bass
import concourse.mybir as mybir
import concourse.tile as tile
from concourse.bass_utils import run_bass_kernel_spmd
from concourse.masks import make_identity

F32 = mybir.dt.float32
BF = mybir.dt.bfloat16  # 1 cyc/row at any N

B, T, D = 4, 2048, 1024
H, DH = 16, 64
HPC = 8          # heads per core
NPAIR = HPC // 2
DCH = D // 128   # 8 contraction chunks
N_CORES = 8
_uid = [0]


def _split_multiwaits(nc, maxw=1):
    """walrus rejects instructions with multiple sync waits (observed on the
    Tile exit drain). Move extra waits onto same-engine NoOps just before."""
    for _name, bbh in nc.bb_map.items():
        bb = bbh.bb if hasattr(bbh, "bb") else bbh
        insts = bb.instructions
        new = []
        for inst in insts:
            si = inst.sync_info
            if si is not None and len(si.on_wait) > maxw:
                waits = list(si.on_wait)
                extra, keep = waits[:-maxw], waits[-maxw:]
                for k in range(0, len(extra), maxw):
                    _uid[0] += 1
                    nop = mybir.InstNoOp(
                        name=f"I-waitsplit-{_uid[0]}", ins=[], outs=[]
                    )
                    nop.engine = inst.engine
                    nop.sync_info = mybir.SyncInfo(
                        on_wait=extra[k : k + maxw], on_update=[]
                    )
                    new.append(nop)
                inst.sync_info = mybir.SyncInfo(
                    on_wait=keep, on_update=list(si.on_update)
                )
            new.append(inst)
        insts[:] = new


def _emit_body(nc, tc, xb, wqk, wv, wout, out):
    Exp = mybir.ActivationFunctionType.Exp
    ts = bass.ts

    from contextlib import ExitStack

    ctx = ExitStack()
    consts = ctx.enter_context(tc.tile_pool(name="consts", bufs=1))
    ident_f = consts.tile([128, 128], F32)
    make_identity(nc, ident_f)
    wout_sb = consts.tile([64, 1024], BF)
    nc.gpsimd.dma_start(out=wout_sb, in_=wout[:])
    v_sb = consts.tile([128, 16, 65], BF)   # v tiles [t,dh] + ones column
    nc.vector.memset(v_sb[:, :, 64:65], 1.0)
    qk_sb = consts.tile([128, 8, T], BF)    # groups: q-pairs 0..3, k-pairs 4..7
    acc = consts.tile([128, 16, 64], F32)   # sum_h attn_h/denom_h, [t, dh]
    nc.vector.memset(acc, 0.0)
    accT = consts.tile([64, 16, 128], BF)
    ot_pool = ctx.enter_context(tc.tile_pool(name="ot", bufs=16))

    sb_ctx = ExitStack()
    xt_pool = sb_ctx.enter_context(tc.tile_pool(name="xt", bufs=1, side="right"))
    w_pool = sb_ctx.enter_context(tc.tile_pool(name="w", bufs=1, side="right"))
    xb_sb = xt_pool.tile([128, DCH, T], BF)
    wqk_sb = w_pool.tile([128, DCH, 1024], BF)
    wv_sb = w_pool.tile([128, DCH, 64], BF)
    # wqk is packed pair-major on the host: pair m at cols [256m, 256m+256)
    # (q then k). Inputs arrive as a few large HWDGE transfers (Pool-issued
    # per-chunk DMAs pay ~1us software-DGE serialization each); pair 0's wqk
    # slice goes ahead of the rest so the first pair unblocks right after xb.
    def r3(ap, p=128):
        return ap.rearrange("(dc p) c -> p dc c", p=p)

    nc.scalar.dma_start(out=wv_sb, in_=r3(wv[:]))
    nc.scalar.dma_start(out=xb_sb[:, 0:4, :], in_=r3(xb[0:512, :]))
    nc.sync.dma_start(out=wqk_sb[:, :, 0:256], in_=r3(wqk[:, 0:256]))
    nc.scalar.dma_start(out=xb_sb[:, 4:8, :], in_=r3(xb[512:1024, :]))
    # The non-pair-0 weights must not cut ahead of xb on the DMA engines
    # (xb gates everything). A tiny Pool copy reading xb into the transfer's
    # destination region forces the DMA (WAW) to request only after xb lands.
    nc.gpsimd.tensor_copy(wqk_sb[:, 7:8, 256:257], xb_sb[:, 7:8, 2047:2048])
    nc.gpsimd.dma_start(out=wqk_sb[:, :, 256:1024], in_=r3(wqk[:, 256:1024]))

    qkctx = ExitStack()
    psqk = qkctx.enter_context(
        tc.tile_pool(name="psqk", bufs=1, space="PSUM", side="right")
    )

    qk0ctx = ExitStack()
    psqk0 = qk0ctx.enter_context(
        tc.tile_pool(name="psqk0", bufs=3, space="PSUM", side="right")
    )

    # ---- v projection: out[t, dh], all 16 t-tiles chained per dc.
    # One accumulation group per PSUM bank (a start wipes the whole bank), so
    # start only on the first touch of each bank, stop on the last.
    psv_ctx = ExitStack()
    psv = psv_ctx.enter_context(
        tc.tile_pool(name="psv", bufs=1, space="PSUM", side="right")
    )
    v_ps = psv.tile([128, 16, 64], F32)

    def emit_vproj(dcs):
        for dc in dcs:
            for tt in range(16):
                nc.tensor.matmul(
                    v_ps[:, tt, :],
                    xb_sb[:, dc, ts(tt, 128)],
                    wv_sb[:, dc, :],
                    start=(dc == 0 and tt % 8 == 0),
                    stop=(dc == DCH - 1 and tt % 8 == 7),
                )



    def emit_qkproj(m, pool=None, use_act=False, tccs=range(4)):
        # project q-pair m (group m) and k-pair m (group 4+m) -> qk_sb bf16.
        # q/k alternate per t-chunk so the first attention rows unblock early.
        # Pair 0 runs before any exp, so its copies can also use the idle Act
        # engine to halve the copy serialization on the critical path.
        for tcc in tccs:
            for g, off in ((m, 256 * m), (4 + m, 256 * m + 128)):
                ps = (pool or psqk).tile([128, 512], F32, tag="qk")
                for dc in range(DCH):
                    nc.tensor.matmul(
                        ps,
                        wqk_sb[:, dc, off : off + 128],
                        xb_sb[:, dc, ts(tcc, 512)],
                        start=(dc == 0),
                        stop=(dc == DCH - 1),
                    )
                if use_act:
                    nc.scalar.copy(qk_sb[:, g, ts(tcc, 512)], ps)
                else:
                    nc.vector.tensor_copy(qk_sb[:, g, ts(tcc, 512)], ps)

    emit_vproj(range(DCH))

    emit_qkproj(0, pool=psqk0, use_act=True)
    # v copy emitted after pair-0's q copies so DVE's in-order stream
    # doesn't head-of-line block the attention-critical copies behind it
    nc.vector.tensor_copy(v_sb[:, :, 0:64], v_ps)
    psv_ctx.close()
    qk0ctx.close()

    at_ctx = ExitStack()
    psS = at_ctx.enter_context(tc.tile_pool(name="psS", bufs=2, space="PSUM"))
    psPV = at_ctx.enter_context(tc.tile_pool(name="psPV", bufs=1, space="PSUM"))
    et_pool = at_ctx.enter_context(tc.tile_pool(name="et", bufs=8))
    nrm_pool = at_ctx.enter_context(tc.tile_pool(name="nrm", bufs=2))

    def emit_attn_pass(m, pass_):
        # The two heads of the pair interleave at row granularity: while the
        # Act engine exponentiates head A's row, PE runs head B's scores/PV.
        # pv holds both heads' accumulators (one bank per head = one
        # accumulation group each); dn shares a single bank as ONE group
        # spanning both heads (disjoint slices).
        if True:
            t0 = 1024 * pass_
            jb = 8 * pass_
            imax = 8 + 8 * pass_
            pv = psPV.tile([128, 2, 8, 64], F32, tag="pv")
            dn = psPV.tile([128, 2, 8], F32, tag="dn")

            def emit_pv_row(ets, i, t_start, off):
                # PV emission lags one row group behind S/exp: keeps PE's
                # in-order stream from head-of-line blocking on the pv slot
                # (only freed by the previous pass's normalize) or on exp(i).
                first = i == 0
                last = i == imax - 1
                for hh in (0, 1):
                    for jj in range(max(i, jb), jb + 8):
                        lo = off + 128 * jj - t_start
                        eti = ets[hh][:, lo : lo + 128]
                        st = first and jj == jb
                        nc.tensor.matmul(
                            pv[:, hh, jj - jb, :],
                            eti,
                            v_sb[:, i, 0:64],
                            start=st,
                            stop=last,
                        )
                        nc.tensor.matmul(
                            dn[:, hh, jj - jb : jj - jb + 1],
                            eti,
                            v_sb[:, i, 64:65],
                            start=(st and hh == 0),
                            stop=(last and hh == 1),
                        )

            def emit_srow(S, et_w, i, t_start, w, off, own, fill_to):
                # scores for row i into S[:, off:off+w]; each PSUM bank of the
                # tile holds exactly one accumulation group. `own` = this row
                # owns bank0 of the tile starting at off=0 (w may span both
                # banks); merged rows sit alone in bank1 (off=512, w<=512).
                lim = min(w, 512)
                p0 = 64 * own
                kTi = qk_sb[p0 : p0 + 64, 4 + m, ts(i, 128)]
                qT = qk_sb[p0 : p0 + 64, m, :]
                junk = fill_to - lim if off == 0 else 0
                nc.tensor.matmul(
                    S[:, off : off + lim],
                    kTi,
                    qT[:, t_start : t_start + lim],
                    start=True,
                    stop=(junk == 0),
                )
                if junk:
                    # merged tiles: pad bank0 to its full span with junk
                    # scores (same accumulation group; exp'd, never read)
                    nc.tensor.matmul(
                        S[:, off + lim : off + fill_to],
                        kTi,
                        qT[:, 0:junk],
                        start=False,
                        stop=True,
                    )
                if w > lim:
                    nc.tensor.matmul(
                        S[:, off + 512 : off + w],
                        kTi,
                        qT[:, t_start + 512 : t_start + w],
                        start=True,
                        stop=True,
                    )

            # rows with w <= 512 pair up two-per-S-tile (one per bank) so a
            # single exp instruction covers both; pairing (a,a+2) keeps the
            # wider row in bank0 with zero wasted exp work.
            if pass_ == 0:
                groups = [[0], [1], [2], [3], [4, 6], [5, 7]]
            else:
                groups = [[i] for i in range(12)] + [[12, 14], [13, 15]]
            pend = []
            for grp in groups:
                rows = []
                for r_idx, i in enumerate(grp):
                    t_start = max(128 * i, t0)
                    rows.append((i, t_start, t0 + 1024 - t_start, 512 * r_idx))
                ets = []
                for hh in (0, 1):
                    S = psS.tile([128, 1024], F32, tag="s")
                    et = et_pool.tile([128, 1024], BF, tag="et")
                    for i, t_start, w, off in rows:
                        ft = min(w, 512) if len(rows) == 1 else 512
                        emit_srow(S, et, i, t_start, w, off, hh, ft)
                    wtot = rows[-1][3] + rows[-1][2]
                    nc.scalar.activation(
                        et[:, 0:wtot], S[:, 0:wtot], Exp, scale=0.125
                    )
                    for i, t_start, w, off in rows:
                        if 128 * i >= t0:
                            # causal diag: zero et where col < row, on the
                            # otherwise-idle Pool engine instead of a PE
                            # additive-mask matmul
                            nc.gpsimd.affine_select(
                                out=et[:, off : off + 128],
                                in_=et[:, off : off + 128],
                                compare_op=mybir.AluOpType.is_ge,
                                fill=0.0,
                                base=0,
                                # keep (col - row) >= 0
                                pattern=[[1, 128]],
                                channel_multiplier=-1,
                            )
                    ets.append(et)
                new = [(ets, i, t_start, off) for i, t_start, w, off in rows]
                for p in pend:
                    emit_pv_row(*p)
                pend = new
            for p in pend:
                emit_pv_row(*p)
            # normalize by the row-sums; accumulate both heads into acc
            rcp = nrm_pool.tile([128, 2, 8], F32, tag="rcp")
            nc.vector.reciprocal(rcp, dn)
            prod = nrm_pool.tile([128, 2, 8, 64], F32, tag="prod")
            nc.vector.tensor_mul(
                prod, pv, rcp.unsqueeze(3).broadcast_to([128, 2, 8, 64])
            )
            sum2 = nrm_pool.tile([128, 8, 64], F32, tag="sum2")
            nc.vector.tensor_add(sum2, prod[:, 0], prod[:, 1])
            nc.vector.tensor_add(
                acc[:, jb : jb + 8, :], acc[:, jb : jb + 8, :], sum2
            )

    def emit_stageD_tt(tt, psD, act_copy):
        pT = psD.tile([64, 128], F32, tag="d")
        nc.tensor.transpose(pT, acc[:, tt, :], ident_f)
        nc.vector.tensor_copy(accT[:, tt, :], pT)
        ot = ot_pool.tile([128, 1024], BF, tag="ot")
        for mc in range(2):
            po = psD.tile([128, 512], F32, tag="d")
            nc.tensor.matmul(
                po,
                accT[:, tt, :],
                wout_sb[:, ts(mc, 512)],
                start=True,
                stop=True,
            )
            if mc == 1 and act_copy:
                nc.scalar.copy(ot[:, 512:1024], po)
            else:
                nc.vector.tensor_copy(ot[:, ts(mc, 512)], po)
        nc.sync.dma_start(out=out[ts(tt, 128), :], in_=ot)

    dA_ctx = ExitStack()
    for m in range(NPAIR):
        emit_attn_pass(m, 0)
        if m + 1 == NPAIR:
            # stage D for the first t-half trickles through the last pair's
            # pass B on the bank freed by psqk (single slot self-throttles).
            qkctx.close()
            sb_ctx.close()
            psD_a = dA_ctx.enter_context(
                tc.tile_pool(name="psDa", bufs=1, space="PSUM", side="right")
            )
        emit_attn_pass(m, 1)
        if m + 1 < NPAIR:
            emit_qkproj(m + 1)
        if m + 1 == NPAIR:
            for tt in range(8):
                emit_stageD_tt(tt, psD_a, act_copy=False)
    dA_ctx.close()
    at_ctx.close()

    # ---- stage D, second t-half: runs at the tail with roomy pools ----
    d_ctx = ExitStack()
    psD_b = d_ctx.enter_context(tc.tile_pool(name="psDb", bufs=8, space="PSUM"))
    for tt in range(8, 16):
        emit_stageD_tt(tt, psD_b, act_copy=True)
    d_ctx.close()
    ctx.close()


_NC_CACHE = [None]


def build_nc():
    if _NC_CACHE[0] is not None:
        return _NC_CACHE[0]
    nc = bass.Bass("TRN2", target_bir_lowering=False, debug=False)
    xb = nc.declare_dram_parameter("xb", [D, T], BF, isOutput=False)
    wqk = nc.declare_dram_parameter("wqk", [D, 1024], BF, isOutput=False)
    wv = nc.declare_dram_parameter("wv", [D, 64], BF, isOutput=False)
    wout = nc.declare_dram_parameter("wout", [64, D], BF, isOutput=False)
    out = nc.declare_dram_parameter("out", [T, D], BF, isOutput=True)
    with tile.TileContext(nc) as tc, nc.allow_low_precision(
        reason="f32r qkv proj + bf16 attention path; ~6e-3 rel err vs fp32 ref"
    ):
        _emit_body(nc, tc, xb, wqk, wv, wout, out)
    _split_multiwaits(nc, maxw=1)
    _NC_CACHE[0] = nc
    return nc


def make_in_maps(x, W_qkv, W_out):
    import ml_dtypes

    bf16 = ml_dtypes.bfloat16
    wv = np.ascontiguousarray(W_qkv[:, 2 * H * DH :]).astype(bf16)
    wout = np.ascontiguousarray(np.asarray(W_out) / float(H)).astype(bf16)
    in_maps = []
    for core in range(N_CORES):
        b, hg = core // 2, core % 2
        xTf = np.ascontiguousarray(np.asarray(x[b]).T, dtype=np.float32)
        cols = []
        for mp in range(NPAIR):  # pair-major: [q-pair | k-pair] per pair
            h0 = hg * HPC + 2 * mp
            for off in (0, H * DH):
                cols.append(W_qkv[:, off + h0 * DH : off + (h0 + 2) * DH])
        wqk = np.ascontiguousarray(np.concatenate(cols, axis=1)).astype(bf16)
        in_maps.append(
            {
                "xb": xTf.astype(bf16),
                "wqk": wqk,
                "wv": wv,
                "wout": wout,
            }
        )
    return in_maps


def kernel(x, W_qkv, W_out, _trace=False, _trace_kwargs=None):
    nc = build_nc()
    in_maps = make_in_maps(x, W_qkv, W_out)
    res = run_bass_kernel_spmd(
        nc, in_maps, list(range(N_CORES)), trace=_trace, **(_trace_kwargs or {})
    )
    out = np.empty((B, T, D), dtype=np.float32)
    for b in range(B):
        out[b] = np.asarray(res.results[2 * b]["out"], dtype=np.float32) + np.asarray(res.results[2 * b + 1]["out"], dtype=np.float32)
    if _trace:
        return out, res
    return out
